# revision 1
# baseline (speedup 1.0000x reference)
"""Self-contained Trainium2 Bass kernel for nn_AttentionBlock_80315888435976.

AttentionBlock: GroupNorm(16 groups) -> 1x1-conv q/k/v -> softmax attention
over the 32x32 spatial grid -> 1x1-conv out-projection -> residual.
Input x: [32, 512, 32, 32] fp32; weights [512, 512]; all biases [512].

Distribution: data-parallel over the batch dim across 8 NeuronCores
(4 batch elements per core); weights broadcast; no collectives.

See build_attention_nc for the device-side formulation and the
performance notes.
"""
import sys
sys.path.insert(0, "/opt/trn_rl_repo")

import contextlib
import numpy as np

import concourse.bass as bass
import concourse.bacc as bacc
import concourse.tile as tile
from concourse import mybir

F32 = mybir.dt.float32
F32R = mybir.dt.float32r
U32 = mybir.dt.uint32
AF = mybir.ActivationFunctionType
OP = mybir.AluOpType

C = 512
N = 1024
G = 16
GW = C // G      # 32 channels per group
CC = C // 128    # 4 channel chunks
NM = N // 128    # 8 m chunks
NH = N // 512    # 2 free halves
EPS = 1e-6
SCALE = 1.0 / np.sqrt(C)
# vecpack columns: 0 gnsc, 1 gnb, 2 bq, 3 bk, 4 beff,
#                  5:21 indm_sums (1/(GW*N)), 21:37 indm_mv (1/GW)
VP = 37
GE = 33        # gse rows: 0..15 = groups, 32 = bias row (base-partition
               # alignment: compute-engine APs must start at multiples of 32)


def build_attention_nc(nbatch=4, mm_dt="f32r", n_cores=8, use_beff=False):
    nc = bacc.Bacc("TRN2", target_bir_lowering=False, debug=False,
                   num_devices=n_cores)
    rdt = F32R if mm_dt == "f32r" else F32

    xs = nc.dram_tensor("xs", [nbatch, C, N], F32, kind="ExternalInput")
    wq = nc.dram_tensor("wqT", [C, C], rdt, kind="ExternalInput")
    wk = nc.dram_tensor("wkT", [C, C], rdt, kind="ExternalInput")
    wv = nc.dram_tensor("wvT", [C, C], rdt, kind="ExternalInput")
    wo = nc.dram_tensor("woT", [C, C], rdt, kind="ExternalInput")
    vpack = nc.dram_tensor("vpack", [C, VP], F32, kind="ExternalInput")
    indT = nc.dram_tensor("indT", [GE, C], F32, kind="ExternalInput")
    onesd = nc.dram_tensor("ones", [128, 1], rdt, kind="ExternalInput")
    outd = nc.dram_tensor("out", [nbatch, C, N], F32, kind="ExternalOutput")

    def r(dram2d):  # [C, X] dram -> [128, CC, X] view
        return dram2d.ap().rearrange("(cc p) x -> p cc x", p=128)

    def mm(ps, lhsT, rhs, start, stop):
        nc.tensor.matmul(ps, lhsT, rhs, start=start, stop=stop)

    with tile.TileContext(nc) as tc, contextlib.ExitStack() as ctx:
        wpool = ctx.enter_context(tc.tile_pool(name="w", bufs=1))
        vecs = ctx.enter_context(tc.tile_pool(name="vecs", bufs=1))
        xpool = ctx.enter_context(tc.tile_pool(name="x", bufs=2))
        hpool = ctx.enter_context(tc.tile_pool(name="hn", bufs=1))
        qkpool = ctx.enter_context(tc.tile_pool(name="qk", bufs=1))
        vpool = ctx.enter_context(tc.tile_pool(name="v", bufs=1))
        epool = ctx.enter_context(tc.tile_pool(name="e", bufs=3))
        upool = ctx.enter_context(tc.tile_pool(name="u", bufs=1))
        opool = ctx.enter_context(tc.tile_pool(name="o", bufs=1))
        rpool = ctx.enter_context(tc.tile_pool(name="r", bufs=2))
        stats = ctx.enter_context(tc.tile_pool(name="st", bufs=2))
        ps_pool = ctx.enter_context(tc.tile_pool(name="ps", bufs=4, space="PSUM"))
        acc_pool = ctx.enter_context(tc.tile_pool(name="acc", bufs=1, space="PSUM"))

        # ---- constants (3 DMAs; HWDGE issue pipe costs ~0.65us per DMA) ----
        vp_sb = vecs.tile([128, CC, VP], F32, tag="vp")
        indT_sb = vecs.tile([GE, CC, 128], F32, tag="indT")
        ones_sb = vecs.tile([128, 1], rdt, tag="ones")
        gse = vecs.tile([GE, 2], F32, tag="gse")
        eps_sb = vecs.tile([G, 1], F32, tag="eps")
        magic_sb = vecs.tile([G, 1], U32, tag="magic")
        c15_sb = vecs.tile([G, 1], F32, tag="c15")
        nc.sync.dma_start(out=vp_sb[:], in_=r(vpack))
        nc.sync.dma_start(
            out=indT_sb[:], in_=indT.ap().rearrange("g (cc p) -> g cc p", p=128))
        nc.sync.dma_start(out=ones_sb[:], in_=onesd.ap())
        nc.vector.memset(eps_sb[:], EPS)
        nc.vector.memset(magic_sb[:], 0x5f3759df)
        nc.vector.memset(c15_sb[:], 1.5)
        nc.vector.memset(gse[32:GE, 0:1], 0.0)
        nc.vector.memset(gse[32:GE, 1:2], 1.0)
        gnsc_sb = vp_sb[:, :, 0:1]
        gnb_sb = vp_sb[:, :, 1:2]
        bq_sb = vp_sb[:, :, 2:3]
        bk_sb = vp_sb[:, :, 3:4]
        beff_sb = vp_sb[:, :, 4:5]

        def stat_op(xt, sums, scr, k):
            """k-th of 8 ACT ops accumulating per-channel sum / sum-sq."""
            cc, which = divmod(k, 2)
            nc.scalar.activation(out=scr[:], in_=xt[:, cc, :],
                                 func=(AF.Copy if which == 0 else AF.Square),
                                 accum_out=sums[:, cc, which:which + 1])

        def gn_stat_tiles():
            return (stats.tile([128, CC, 2], F32, tag="sums", name="sums"),
                    stats.tile([128, N], F32, tag="scr", name="scr"))

        def gn_sum_mms(sums, dve_chunks=()):
            ps_g = ps_pool.tile([G, 2], F32, tag="ps")
            for cc in range(CC):
                col = slice(21, 37) if cc in dve_chunks else slice(5, 21)
                nc.tensor.matmul(ps_g[:], vp_sb[:, cc, col], sums[:, cc, :],
                                 start=(cc == 0), stop=(cc == CC - 1))
            return ps_g

        # ---- batch-0 x load: per chunk, stats split ACT/DVE ----
        xt0 = xpool.tile([128, CC, N], F32, tag="x", name="xt0")
        sums0, scr0 = gn_stat_tiles()
        st6_0 = stats.tile([128, CC, 2, 6], F32, tag="st6")
        mv0 = stats.tile([128, CC, 2], F32, tag="mv")
        b0_dve_chunks = (1, 3)
        for cc in range(CC):
            nc.sync.dma_start(out=xt0[:, cc, :],
                              in_=xs.ap()[0][bass.ts(cc, 128), :])
            if cc in b0_dve_chunks:
                # DVE path -> sums0[:, cc] = [mu_c, mu_c^2 + var_c]
                for h in range(2):
                    nc.vector.bn_stats(out=st6_0[:, cc, h, :],
                                       in_=xt0[:, cc, bass.ts(h, 512)])
                nc.vector.bn_aggr(out=mv0[:, cc, :], in_=st6_0[:, cc, :, :])
                nc.vector.tensor_mul(out=sums0[:, cc, 1:2],
                                     in0=mv0[:, cc, 0:1], in1=mv0[:, cc, 0:1])
                nc.vector.tensor_add(out=sums0[:, cc, 1:2],
                                     in0=sums0[:, cc, 1:2], in1=mv0[:, cc, 1:2])
                nc.vector.tensor_copy(out=sums0[:, cc, 0:1],
                                      in_=mv0[:, cc, 0:1])
            else:
                stat_op(xt0, sums0, scr0, 2 * cc)
                stat_op(xt0, sums0, scr0, 2 * cc + 1)

        wq_sb = wpool.tile([128, CC, C], rdt, tag="wq")
        wk_sb = wpool.tile([128, CC, C], rdt, tag="wk")
        wv_sb = wpool.tile([128, CC, C], rdt, tag="wv")
        wo_sb = wpool.tile([128, CC, C], rdt, tag="wo")
        nc.sync.dma_start(out=wq_sb[:], in_=r(wq))
        nc.sync.dma_start(out=wk_sb[:], in_=r(wk))
        nc.sync.dma_start(out=wv_sb[:], in_=r(wv))
        nc.sync.dma_start(out=wo_sb[:], in_=r(wo))

        def load_x(b):
            xt = xpool.tile([128, CC, N], F32, tag="x")
            nc.sync.dma_start(
                out=xt[:], in_=xs.ap()[b].rearrange("(cc p) n -> p cc n", p=128))
            return xt

        def gn_finish(ps_g):
            """[mu_g, m2_g] -> gse rows 0..15 = [rstd_g, -mu_g*rstd_g]."""
            gsb = stats.tile([G, 2], F32, tag="gsb")
            varg = stats.tile([G, 1], F32, tag="varg")
            nc.vector.tensor_copy(out=gsb[:], in_=ps_g[:])
            nc.vector.tensor_mul(out=varg[:], in0=gsb[:, 0:1], in1=gsb[:, 0:1])
            nc.vector.tensor_tensor(out=varg[:], in0=gsb[:, 1:2], in1=varg[:],
                                    op=OP.subtract)
            nc.vector.tensor_scalar_add(out=varg[:], in0=varg[:], scalar1=EPS)
            y = stats.tile([G, 1], F32, tag="nwt_y")
            vh = stats.tile([G, 1], F32, tag="nwt_vh")
            t = stats.tile([G, 1], F32, tag="nwt_t")
            nc.vector.tensor_scalar(out=t[:].bitcast(U32),
                                    in0=varg[:].bitcast(U32),
                                    scalar1=1, scalar2=None,
                                    op0=OP.logical_shift_right)
            nc.vector.tensor_tensor(out=y[:].bitcast(U32), in0=magic_sb[:],
                                    in1=t[:].bitcast(U32), op=OP.subtract)
            nc.vector.tensor_scalar_mul(out=vh[:], in0=varg[:], scalar1=0.5)
            for it in range(2):
                nc.vector.tensor_mul(out=t[:], in0=y[:], in1=y[:])
                nc.vector.tensor_mul(out=t[:], in0=vh[:], in1=t[:])
                nc.vector.tensor_tensor(out=t[:], in0=c15_sb[:], in1=t[:],
                                        op=OP.subtract)
                dst = gse[0:G, 0:1] if it == 1 else y[:]
                nc.vector.tensor_mul(out=dst, in0=y[:], in1=t[:])
            nc.vector.tensor_mul(out=t[:], in0=gsb[:, 0:1], in1=gse[0:G, 0:1])
            nc.vector.tensor_scalar_mul(out=gse[0:G, 1:2], in0=t[:],
                                        scalar1=-1.0)

        def gn_ab():
            ab_sb = stats.tile([128, CC, 2], F32, tag="ab_sb")
            for cc in range(CC):
                ps_cb = ps_pool.tile([128, 2], F32, tag="ps")
                nc.tensor.matmul(ps_cb[:], indT_sb[:, cc, :], gse[:],
                                 start=True, stop=True)
                nc.scalar.activation(out=ab_sb[:, cc, :], in_=ps_cb[:],
                                     func=AF.Copy)
            return ab_sb

        def gn_hn_apply(xt, ab_sb, hn, cc):
            nc.scalar.activation(out=hn[:, cc, :], in_=xt[:, cc, :],
                                 func=AF.Identity,
                                 scale=ab_sb[:, cc, 0:1],
                                 bias=ab_sb[:, cc, 1:2])

        def gn_apply(xt):
            ab_sb = gn_ab()
            hn = hpool.tile([128, CC, N], rdt, tag="hn")
            for cc in range(CC):
                gn_hn_apply(xt, ab_sb, hn, cc)
            return hn

        def qkv(hn):
            qt = qkpool.tile([128, CC, N], rdt, tag="q")
            kt = qkpool.tile([128, CC, N], rdt, tag="k")
            for wsb, bias_sb, dst in ((wq_sb, bq_sb, qt), (wk_sb, bk_sb, kt)):
                for co in range(CC):
                    for h in range(NH):
                        ps_t = ps_pool.tile([128, 512], F32, tag="ps")
                        for ci in range(CC):
                            mm(ps_t[:], wsb[:, ci, bass.ts(co, 128)],
                               hn[:, ci, bass.ts(h, 512)], ci == 0, ci == CC - 1)
                        nc.vector.tensor_scalar_add(
                            out=dst[:, co, bass.ts(h, 512)], in0=ps_t[:],
                            scalar1=bias_sb[:, co, :])
            vT = vpool.tile([128, NM, C], rdt, tag="vT")
            for mo in range(NM):
                ps_t = ps_pool.tile([128, 512], F32, tag="ps")
                for ci in range(CC):
                    mm(ps_t[:], hn[:, ci, bass.ts(mo, 128)], wv_sb[:, ci, :],
                       ci == 0, ci == CC - 1)
                nc.vector.tensor_copy(out=vT[:, mo, :], in_=ps_t[:])
            return qt, kt, vT

        def attention(qt, kt, vT, next_xt=None):
            Zs = rpool.tile([1, N], F32, tag="Zs")
            Zb = rpool.tile([128, N], F32, tag="Zb")
            U = upool.tile([128, CC, N], rdt, tag="U")
            sums_next = scr_next = hn_next = None
            ab_next = [None]
            if next_xt is not None:
                sums_next, scr_next = gn_stat_tiles()
                hn_next = hpool.tile([128, CC, N], rdt, tag="hn", name="hn")
            for h in range(NH):
                ps_z = ps_pool.tile([1, 512], F32, tag="ps")
                ps_u = acc_pool.tile([128, CC, 512], F32, tag="acc")
                for mo in range(NM):
                    ps_s = ps_pool.tile([128, 512], F32, tag="ps")
                    for ci in range(CC):
                        mm(ps_s[:], kt[:, ci, bass.ts(mo, 128)],
                           qt[:, ci, bass.ts(h, 512)], ci == 0, ci == CC - 1)
                    ech = epool.tile([128, 512], rdt, tag="e")
                    nc.scalar.activation(out=ech[:], in_=ps_s[:], func=AF.Exp,
                                         scale=SCALE)
                    mm(ps_z[:], ones_sb[:], ech[:], mo == 0, mo == NM - 1)
                    for co in range(CC):
                        mm(ps_u[:, co, :], vT[:, mo, bass.ts(co, 128)], ech[:],
                           mo == 0, mo == NM - 1)
                    if h == 0 and next_xt is not None:
                        stat_op(next_xt, sums_next, scr_next, mo)
                    if h == 1 and next_xt is not None:
                        if mo == 0:
                            ps_g = gn_sum_mms(sums_next)
                            gn_finish(ps_g)
                        elif mo == 2:
                            ab_next[0] = gn_ab()
                        elif mo >= 4:
                            gn_hn_apply(next_xt, ab_next[0], hn_next, mo - 4)
                sl = bass.ts(h, 512)
                if h == 1:
                    nc.scalar.activation(out=Zs[:, sl], in_=ps_z[:],
                                         func=AF.Copy)
                else:
                    nc.vector.tensor_copy(out=Zs[:, sl], in_=ps_z[:])
                nc.gpsimd.partition_broadcast(Zb[:, sl], Zs[:, sl])
                nc.vector.reciprocal(out=Zb[:, sl], in_=Zb[:, sl])
                for co in range(CC):
                    if h == 1 or co % 2:
                        nc.scalar.activation(out=U[:, co, bass.ts(h, 512)],
                                             in_=ps_u[:, co, :], func=AF.Copy)
                    else:
                        nc.vector.tensor_copy(out=U[:, co, bass.ts(h, 512)],
                                              in_=ps_u[:, co, :])
            return U, Zb, hn_next

        def proj_mms(U, Zb):
            out_sb = opool.tile([128, CC, N], F32, tag="out")
            for h in range(NH):
                sl = bass.ts(h, 512)
                for co in range(CC):
                    ps_o = ps_pool.tile([128, 512], F32, tag="ps")
                    for ci in range(CC):
                        mm(ps_o[:], wo_sb[:, ci, bass.ts(co, 128)],
                           U[:, ci, sl], ci == 0, ci == CC - 1)
                    nc.vector.tensor_mul(out=out_sb[:, co, sl],
                                         in0=ps_o[:], in1=Zb[:, sl])
            return out_sb

        def epilogue(out_sb, xt, b):
            for h in range(NH):
                sl = bass.ts(h, 512)
                for co in range(CC):
                    nc.vector.tensor_add(out=out_sb[:, co, sl],
                                         in0=out_sb[:, co, sl],
                                         in1=xt[:, co, sl])
                    if use_beff:
                        nc.vector.tensor_scalar_add(out=out_sb[:, co, sl],
                                                    in0=out_sb[:, co, sl],
                                                    scalar1=beff_sb[:, co, :])
                nc.gpsimd.dma_start(
                    out=outd.ap()[b].rearrange("(cc p) n -> p cc n",
                                               p=128)[:, :, sl],
                    in_=out_sb[:, :, sl])

        # ---- software-pipelined batch loop ----
        # GN of batch b+1 (stats, group matmuls, Newton rsqrt, broadcast,
        # affine apply) is emitted INSIDE attention(b), where PE/ACT/DVE
        # all have slack; batch boundaries carry only proj -> qkv.
        pending = None
        xt_cur = xt0
        hn_cur = None
        for b in range(nbatch):
            if b == 0:
                ps_g = gn_sum_mms(sums0, dve_chunks=b0_dve_chunks)
                gn_finish(ps_g)
                hn_cur = gn_apply(xt_cur)
            out_prev = None
            if pending is not None:
                out_prev = proj_mms(pending[0], pending[1])
                epilogue(out_prev, pending[2], pending[3])
            qt, kt, vT = qkv(hn_cur)
            xt_next = load_x(b + 1) if b + 1 < nbatch else None
            U, Zb, hn_next = attention(qt, kt, vT, next_xt=xt_next)
            pending = (U, Zb, xt_cur, b)
            xt_cur = xt_next
            hn_cur = hn_next
        out_last = proj_mms(pending[0], pending[1])
        epilogue(out_last, pending[2], pending[3])

    nc.compile()
    return nc


def make_host_inputs(x, gn_scale, gn_bias, wq, bq, wk, bk, wv, bv, wo, bo,
                     n_cores=8):
    """Shard + precompute host-side arrays. Returns (in_maps, nbatch)."""
    B = x.shape[0]
    nbatch = B // n_cores
    xr = np.ascontiguousarray(np.asarray(x, np.float32).reshape(B, C, N))
    beff = (np.asarray(wo, np.float32) @ np.asarray(bv, np.float32)
            + np.asarray(bo, np.float32))
    vpack = np.zeros((C, VP), np.float32)
    vpack[:, 0] = np.asarray(gn_scale, np.float32)
    vpack[:, 1] = np.asarray(gn_bias, np.float32)
    vpack[:, 2] = np.asarray(bq, np.float32)
    vpack[:, 3] = np.asarray(bk, np.float32)
    vpack[:, 4] = beff
    cidx = np.arange(C)
    vpack[cidx, 5 + cidx // GW] = 1.0 / (GW * N)
    vpack[cidx, 21 + cidx // GW] = 1.0 / GW
    indT = np.zeros((33, C), np.float32)
    indT[cidx // GW, cidx] = np.asarray(gn_scale, np.float32)
    indT[32, :] = np.asarray(gn_bias, np.float32)
    common = {
        "wqT": np.ascontiguousarray(np.asarray(wq, np.float32).T),
        "wkT": np.ascontiguousarray(np.asarray(wk, np.float32).T),
        "wvT": np.ascontiguousarray(np.asarray(wv, np.float32).T),
        "woT": np.ascontiguousarray(np.asarray(wo, np.float32).T),
        "vpack": vpack,
        "indT": indT,
        "ones": np.ones((128, 1), np.float32),
    }
    in_maps = []
    for i in range(n_cores):
        m = dict(common)
        m["xs"] = np.ascontiguousarray(xr[i * nbatch:(i + 1) * nbatch])
        in_maps.append(m)
    return in_maps, nbatch


_NC_CACHE = {}


def _get_nc(nbatch, use_beff):
    key = (nbatch, use_beff)
    if key not in _NC_CACHE:
        _NC_CACHE[key] = build_attention_nc(nbatch=nbatch, mm_dt="f32r",
                                            n_cores=8, use_beff=use_beff)
    return _NC_CACHE[key]


def kernel(x, gn_scale, gn_bias, wq, bq, wk, bk, wv, bv, wo, bo):
    """Full-input entry point: shards over 8 NeuronCores, returns full out."""
    from concourse.bass_utils import run_bass_kernel_spmd

    x = np.asarray(x, np.float32)
    B, Cin, H, W = x.shape
    assert (Cin, H * W) == (C, N), f"unexpected shape {x.shape}"
    n_cores = 8
    assert B % n_cores == 0
    in_maps, nbatch = make_host_inputs(
        x.reshape(B, C, N), gn_scale, gn_bias, wq, bq, wk, bk, wv, bv, wo, bo,
        n_cores=n_cores)
    use_beff = bool(np.any(in_maps[0]["vpack"][:, 4]))
    nc = _get_nc(nbatch, use_beff)
    res = run_bass_kernel_spmd(nc, in_maps, core_ids=list(range(n_cores)))
    out = np.concatenate([res.results[i]["out"] for i in range(n_cores)],
                         axis=0)
    return out.reshape(B, Cin, H, W).astype(np.float32)


# revision 2
# speedup vs baseline: 1.0385x; 1.0385x over previous
"""Self-contained Trainium2 Bass kernel for nn_AttentionBlock_80315888435976.

AttentionBlock: GroupNorm(16 groups) -> 1x1-conv q/k/v -> softmax attention
over the 32x32 spatial grid -> 1x1-conv out-projection -> residual.
Input x: [32, 512, 32, 32] fp32; weights [512, 512]; all biases [512].

Distribution: data-parallel over the batch dim across 8 NeuronCores
(4 batch elements per core); weights broadcast; no collectives.
"""
import sys
sys.path.insert(0, "/opt/trn_rl_repo")

import contextlib
import numpy as np

import concourse.bass as bass
import concourse.bacc as bacc
import concourse.tile as tile
from concourse import mybir

F32 = mybir.dt.float32
F32R = mybir.dt.float32r
U32 = mybir.dt.uint32
AF = mybir.ActivationFunctionType
OP = mybir.AluOpType

C = 512
N = 1024
G = 16
GW = C // G      # 32 channels per group
CC = C // 128    # 4 channel chunks
NM = N // 128    # 8 m chunks
NH = N // 512    # 2 free halves
EPS = 1e-6
SCALE = 1.0 / np.sqrt(C)
# vecpack columns: 0 gnsc, 1 gnb, 2 bq, 3 bk, 4 beff,
#                  5:21 indm_sums (1/(GW*N)), 21:37 indm_mv (1/GW)
VP = 37
GE = 33        # gse rows: 0..15 = groups, 32 = bias row (base-partition
               # alignment: compute-engine APs must start at multiples of 32)


def build_attention_nc(nbatch=4, mm_dt="f32r", n_cores=8, use_beff=False):
    nc = bacc.Bacc("TRN2", target_bir_lowering=False, debug=False,
                   num_devices=n_cores)
    rdt = F32R if mm_dt == "f32r" else F32

    xs = nc.dram_tensor("xs", [nbatch, C, N], F32, kind="ExternalInput")
    wq = nc.dram_tensor("wqT", [C, C], rdt, kind="ExternalInput")
    wk = nc.dram_tensor("wkT", [C, C], rdt, kind="ExternalInput")
    wv = nc.dram_tensor("wvT", [C, C], rdt, kind="ExternalInput")
    wo = nc.dram_tensor("woT", [C, C], rdt, kind="ExternalInput")
    vpack = nc.dram_tensor("vpack", [C, VP], F32, kind="ExternalInput")
    indT = nc.dram_tensor("indT", [GE, C], F32, kind="ExternalInput")
    onesd = nc.dram_tensor("ones", [128, 1], rdt, kind="ExternalInput")
    outd = nc.dram_tensor("out", [nbatch, C, N], F32, kind="ExternalOutput")

    def r(dram2d):  # [C, X] dram -> [128, CC, X] view
        return dram2d.ap().rearrange("(cc p) x -> p cc x", p=128)

    def mm(ps, lhsT, rhs, start, stop):
        nc.tensor.matmul(ps, lhsT, rhs, start=start, stop=stop)

    with tile.TileContext(nc) as tc, contextlib.ExitStack() as ctx:
        wpool = ctx.enter_context(tc.tile_pool(name="w", bufs=1))
        vecs = ctx.enter_context(tc.tile_pool(name="vecs", bufs=1))
        xpool = ctx.enter_context(tc.tile_pool(name="x", bufs=3))
        hpool = ctx.enter_context(tc.tile_pool(name="hn", bufs=1))
        qkpool = ctx.enter_context(tc.tile_pool(name="qk", bufs=1))
        vpool = ctx.enter_context(tc.tile_pool(name="v", bufs=1))
        epool = ctx.enter_context(tc.tile_pool(name="e", bufs=3))
        upool = ctx.enter_context(tc.tile_pool(name="u", bufs=1))
        opool = ctx.enter_context(tc.tile_pool(name="o", bufs=1))
        rpool = ctx.enter_context(tc.tile_pool(name="r", bufs=2))
        stats = ctx.enter_context(tc.tile_pool(name="st", bufs=2))
        ps_pool = ctx.enter_context(tc.tile_pool(name="ps", bufs=4, space="PSUM"))
        acc_pool = ctx.enter_context(tc.tile_pool(name="acc", bufs=1, space="PSUM"))

        # ---- constants (3 DMAs; HWDGE issue pipe costs ~0.65us per DMA) ----
        vp_sb = vecs.tile([128, CC, VP], F32, tag="vp")
        indT_sb = vecs.tile([GE, CC, 128], F32, tag="indT")
        ones_sb = vecs.tile([128, 1], rdt, tag="ones")
        gse = vecs.tile([GE, 2], F32, tag="gse")
        eps_sb = vecs.tile([G, 1], F32, tag="eps")
        magic_sb = vecs.tile([G, 1], U32, tag="magic")
        c15_sb = vecs.tile([G, 1], F32, tag="c15")
        nc.sync.dma_start(out=vp_sb[:], in_=r(vpack))
        nc.vector.memset(eps_sb[:], EPS)
        nc.vector.memset(magic_sb[:], 0x5f3759df)
        nc.vector.memset(c15_sb[:], 1.5)
        nc.vector.memset(gse[32:GE, 0:1], 0.0)
        nc.vector.memset(gse[32:GE, 1:2], 1.0)
        gnsc_sb = vp_sb[:, :, 0:1]
        gnb_sb = vp_sb[:, :, 1:2]
        bq_sb = vp_sb[:, :, 2:3]
        bk_sb = vp_sb[:, :, 3:4]
        beff_sb = vp_sb[:, :, 4:5]

        def stat_op(xt, sums, scr, k):
            """k-th of 8 ACT ops accumulating per-channel sum / sum-sq.
            scr is a scratch dummy output (only accum_out matters)."""
            cc, which = divmod(k, 2)
            nc.scalar.activation(out=scr[:, cc, :],
                                 in_=xt[:, cc, :],
                                 func=(AF.Copy if which == 0 else AF.Square),
                                 accum_out=sums[:, cc, which:which + 1])

        def gn_stat_tiles():
            # (scr is not allocated here: the next batch's hn tile doubles as
            # the dummy activation output until its real write in h1)
            return stats.tile([128, CC, 2], F32, tag="sums", name="sums")

        def gn_sum_mms(sums, dve_chunks=()):
            ps_g = ps_pool.tile([G, 2], F32, tag="ps")
            for cc in range(CC):
                col = slice(21, 37) if cc in dve_chunks else slice(5, 21)
                nc.tensor.matmul(ps_g[:], vp_sb[:, cc, col], sums[:, cc, :],
                                 start=(cc == 0), stop=(cc == CC - 1))
            return ps_g

        # ---- batch-0 x load: per chunk, stats split ACT/DVE ----
        xt0 = xpool.tile([128, CC, N], F32, tag="x", name="xt0")
        hn0 = hpool.tile([128, CC, N], F32R if mm_dt == "f32r" else F32,
                         tag="hn", name="hn0")
        sums0 = gn_stat_tiles()
        st6_0 = stats.tile([128, CC, 2, 6], F32, tag="st6")
        mv0 = stats.tile([128, CC, 2], F32, tag="mv")
        b0_dve_chunks = (1, 3)
        for cc in range(CC):
            nc.sync.dma_start(out=xt0[:, cc, :],
                              in_=xs.ap()[0][bass.ts(cc, 128), :])
            if cc in b0_dve_chunks:
                # DVE path -> sums0[:, cc] = [mu_c, mu_c^2 + var_c]
                for h in range(2):
                    nc.vector.bn_stats(out=st6_0[:, cc, h, :],
                                       in_=xt0[:, cc, bass.ts(h, 512)])
                nc.vector.bn_aggr(out=mv0[:, cc, :], in_=st6_0[:, cc, :, :])
                nc.vector.tensor_mul(out=sums0[:, cc, 1:2],
                                     in0=mv0[:, cc, 0:1], in1=mv0[:, cc, 0:1])
                nc.vector.tensor_add(out=sums0[:, cc, 1:2],
                                     in0=sums0[:, cc, 1:2], in1=mv0[:, cc, 1:2])
                nc.vector.tensor_copy(out=sums0[:, cc, 0:1],
                                      in_=mv0[:, cc, 0:1])
            else:
                stat_op(xt0, sums0, hn0, 2 * cc)
                stat_op(xt0, sums0, hn0, 2 * cc + 1)

        nc.sync.dma_start(
            out=indT_sb[:], in_=indT.ap().rearrange("g (cc p) -> g cc p", p=128))
        nc.sync.dma_start(out=ones_sb[:], in_=onesd.ap())

        wq_sb = wpool.tile([128, CC, C], rdt, tag="wq")
        wk_sb = wpool.tile([128, CC, C], rdt, tag="wk")
        wv_sb = wpool.tile([128, CC, C], rdt, tag="wv")
        wo_sb = wpool.tile([128, CC, C], rdt, tag="wo")
        nc.sync.dma_start(out=wq_sb[:], in_=r(wq))
        nc.sync.dma_start(out=wk_sb[:], in_=r(wk))
        nc.sync.dma_start(out=wv_sb[:], in_=r(wv))
        nc.sync.dma_start(out=wo_sb[:], in_=r(wo))

        def load_x(b):
            xt = xpool.tile([128, CC, N], F32, tag="x")
            nc.sync.dma_start(
                out=xt[:], in_=xs.ap()[b].rearrange("(cc p) n -> p cc n", p=128))
            return xt

        def gn_finish(ps_g):
            """[mu_g, m2_g] -> gse rows 0..15 = [rstd_g, -mu_g*rstd_g]."""
            gsb = stats.tile([G, 2], F32, tag="gsb")
            varg = stats.tile([G, 1], F32, tag="varg")
            nc.vector.tensor_copy(out=gsb[:], in_=ps_g[:])
            nc.vector.tensor_mul(out=varg[:], in0=gsb[:, 0:1], in1=gsb[:, 0:1])
            nc.vector.tensor_tensor(out=varg[:], in0=gsb[:, 1:2], in1=varg[:],
                                    op=OP.subtract)
            nc.vector.tensor_scalar_add(out=varg[:], in0=varg[:], scalar1=EPS)
            y = stats.tile([G, 1], F32, tag="nwt_y")
            vh = stats.tile([G, 1], F32, tag="nwt_vh")
            t = stats.tile([G, 1], F32, tag="nwt_t")
            nc.vector.tensor_scalar(out=t[:].bitcast(U32),
                                    in0=varg[:].bitcast(U32),
                                    scalar1=1, scalar2=None,
                                    op0=OP.logical_shift_right)
            nc.vector.tensor_tensor(out=y[:].bitcast(U32), in0=magic_sb[:],
                                    in1=t[:].bitcast(U32), op=OP.subtract)
            nc.vector.tensor_scalar_mul(out=vh[:], in0=varg[:], scalar1=0.5)
            for it in range(2):
                nc.vector.tensor_mul(out=t[:], in0=y[:], in1=y[:])
                nc.vector.tensor_mul(out=t[:], in0=vh[:], in1=t[:])
                nc.vector.tensor_tensor(out=t[:], in0=c15_sb[:], in1=t[:],
                                        op=OP.subtract)
                dst = gse[0:G, 0:1] if it == 1 else y[:]
                nc.vector.tensor_mul(out=dst, in0=y[:], in1=t[:])
            nc.vector.tensor_mul(out=t[:], in0=gsb[:, 0:1], in1=gse[0:G, 0:1])
            nc.vector.tensor_scalar_mul(out=gse[0:G, 1:2], in0=t[:],
                                        scalar1=-1.0)

        def gn_ab():
            ab_sb = stats.tile([128, CC, 2], F32, tag="ab_sb")
            for cc in range(CC):
                ps_cb = ps_pool.tile([128, 2], F32, tag="ps")
                nc.tensor.matmul(ps_cb[:], indT_sb[:, cc, :], gse[:],
                                 start=True, stop=True)
                nc.scalar.activation(out=ab_sb[:, cc, :], in_=ps_cb[:],
                                     func=AF.Copy)
            return ab_sb

        def gn_hn_apply(xt, ab_sb, hn, cc):
            if cc % 2:
                nc.vector.tensor_scalar(out=hn[:, cc, :], in0=xt[:, cc, :],
                                        scalar1=ab_sb[:, cc, 0:1],
                                        scalar2=ab_sb[:, cc, 1:2],
                                        op0=OP.mult, op1=OP.add)
            else:
                nc.scalar.activation(out=hn[:, cc, :], in_=xt[:, cc, :],
                                     func=AF.Identity,
                                     scale=ab_sb[:, cc, 0:1],
                                     bias=ab_sb[:, cc, 1:2])

        def gn_apply(xt, hn):
            ab_sb = gn_ab()
            for cc in range(CC):
                gn_hn_apply(xt, ab_sb, hn, cc)
            return hn

        def qkv(hn):
            qt = qkpool.tile([128, CC, N], rdt, tag="q")
            kt = qkpool.tile([128, CC, N], rdt, tag="k")
            for wsb, bias_sb, dst in ((wq_sb, bq_sb, qt), (wk_sb, bk_sb, kt)):
                for co in range(CC):
                    for h in range(NH):
                        ps_t = ps_pool.tile([128, 512], F32, tag="ps")
                        for ci in range(CC):
                            mm(ps_t[:], wsb[:, ci, bass.ts(co, 128)],
                               hn[:, ci, bass.ts(h, 512)], ci == 0, ci == CC - 1)
                        nc.vector.tensor_scalar_add(
                            out=dst[:, co, bass.ts(h, 512)], in0=ps_t[:],
                            scalar1=bias_sb[:, co, :])
            vT = vpool.tile([128, NM, C], rdt, tag="vT")
            for mo in range(NM):
                ps_t = ps_pool.tile([128, 512], F32, tag="ps")
                for ci in range(CC):
                    mm(ps_t[:], hn[:, ci, bass.ts(mo, 128)], wv_sb[:, ci, :],
                       ci == 0, ci == CC - 1)
                nc.vector.tensor_copy(out=vT[:, mo, :], in_=ps_t[:])
            return qt, kt, vT

        def attention(qt, kt, vT, next_xt=None):
            Zs = rpool.tile([1, N], F32, tag="Zs")
            Zb = rpool.tile([128, N], F32, tag="Zb")
            U = upool.tile([128, CC, N], rdt, tag="U")
            sums_next = hn_next = None
            ab_next = [None]
            if next_xt is not None:
                sums_next = gn_stat_tiles()
                hn_next = hpool.tile([128, CC, N], rdt, tag="hn", name="hn")
            def emit_scores(h, mo):
                ps_s = ps_pool.tile([128, 512], F32, tag="ps", name="ps_s")
                for ci in range(CC):
                    mm(ps_s[:], kt[:, ci, bass.ts(mo, 128)],
                       qt[:, ci, bass.ts(h, 512)], ci == 0, ci == CC - 1)
                ech = epool.tile([128, 512], rdt, tag="e", name="ech")
                nc.scalar.activation(out=ech[:], in_=ps_s[:], func=AF.Exp,
                                     scale=SCALE)
                return ech

            ech = emit_scores(0, 0)
            for h in range(NH):
                ps_z = ps_pool.tile([1, 512], F32, tag="ps")
                ps_u = acc_pool.tile([128, CC, 512], F32, tag="acc")
                for mo in range(NM):
                    # one-group score lookahead keeps PE busy across the
                    # exp latency and the half-boundary U-psum handoff
                    ech_next = None
                    if mo + 1 < NM:
                        ech_next = emit_scores(h, mo + 1)
                    elif h + 1 < NH:
                        ech_next = emit_scores(h + 1, 0)
                    mm(ps_z[:], ones_sb[:], ech[:], mo == 0, mo == NM - 1)
                    for co in range(CC):
                        mm(ps_u[:, co, :], vT[:, mo, bass.ts(co, 128)], ech[:],
                           mo == 0, mo == NM - 1)
                    ech = ech_next
                    if h == 0 and next_xt is not None:
                        stat_op(next_xt, sums_next, hn_next, mo)
                    if h == 1 and next_xt is not None:
                        if mo == 0:
                            ps_g = gn_sum_mms(sums_next)
                            gn_finish(ps_g)
                        elif mo == 2:
                            ab_next[0] = gn_ab()
                        elif mo >= 4:
                            gn_hn_apply(next_xt, ab_next[0], hn_next, mo - 4)
                sl = bass.ts(h, 512)
                if h == 0:
                    # U copies first: they gate the next half's U accumulation
                    for co in range(CC):
                        if co % 2:
                            nc.scalar.activation(out=U[:, co, sl],
                                                 in_=ps_u[:, co, :],
                                                 func=AF.Copy)
                        else:
                            nc.vector.tensor_copy(out=U[:, co, sl],
                                                  in_=ps_u[:, co, :])
                    nc.vector.tensor_copy(out=Zs[:, sl], in_=ps_z[:])
                else:
                    nc.scalar.activation(out=Zs[:, sl], in_=ps_z[:],
                                         func=AF.Copy)
                    for co in range(CC):
                        nc.scalar.activation(out=U[:, co, sl],
                                             in_=ps_u[:, co, :], func=AF.Copy)
                nc.gpsimd.partition_broadcast(Zb[:, sl], Zs[:, sl])
                nc.vector.reciprocal(out=Zb[:, sl], in_=Zb[:, sl])
            return U, Zb, hn_next

        def proj_mms(U, Zb, fuse=None):
            out_sb = opool.tile([128, CC, N], F32, tag="out")
            for h in range(NH):
                sl = bass.ts(h, 512)
                for co in range(CC):
                    ps_o = ps_pool.tile([128, 512], F32, tag="ps")
                    for ci in range(CC):
                        mm(ps_o[:], wo_sb[:, ci, bass.ts(co, 128)],
                           U[:, ci, sl], ci == 0, ci == CC - 1)
                    nc.vector.tensor_mul(out=out_sb[:, co, sl],
                                         in0=ps_o[:], in1=Zb[:, sl])
                    if fuse is not None:
                        nc.vector.tensor_add(out=out_sb[:, co, sl],
                                             in0=out_sb[:, co, sl],
                                             in1=fuse[:, co, sl])
                        if use_beff:
                            nc.vector.tensor_scalar_add(
                                out=out_sb[:, co, sl],
                                in0=out_sb[:, co, sl],
                                scalar1=beff_sb[:, co, :])
                if fuse is not None:
                    nc.gpsimd.dma_start(
                        out=outd.ap()[fuse_b[0]].rearrange(
                            "(cc p) n -> p cc n", p=128)[:, :, sl],
                        in_=out_sb[:, :, sl])
            return out_sb

        def epilogue(out_sb, xt, b):
            for h in range(NH):
                sl = bass.ts(h, 512)
                for co in range(CC):
                    nc.vector.tensor_add(out=out_sb[:, co, sl],
                                         in0=out_sb[:, co, sl],
                                         in1=xt[:, co, sl])
                    if use_beff:
                        nc.vector.tensor_scalar_add(out=out_sb[:, co, sl],
                                                    in0=out_sb[:, co, sl],
                                                    scalar1=beff_sb[:, co, :])
                nc.gpsimd.dma_start(
                    out=outd.ap()[b].rearrange("(cc p) n -> p cc n",
                                               p=128)[:, :, sl],
                    in_=out_sb[:, :, sl])

        # ---- software-pipelined batch loop ----
        # GN of batch b+1 (stats, group matmuls, Newton rsqrt, broadcast,
        # affine apply) is emitted INSIDE attention(b), where PE/ACT/DVE
        # all have slack; batch boundaries carry only proj -> qkv.
        pending = None
        xt_cur = xt0
        hn_cur = None
        fuse_b = [None]
        for b in range(nbatch):
            if b == 0:
                ps_g = gn_sum_mms(sums0, dve_chunks=b0_dve_chunks)
                gn_finish(ps_g)
                hn_cur = gn_apply(xt_cur, hn0)
            out_prev = None
            if pending is not None:
                out_prev = proj_mms(pending[0], pending[1])
            qt, kt, vT = qkv(hn_cur)
            if pending is not None:
                epilogue(out_prev, pending[2], pending[3])
            xt_next = load_x(b + 1) if b + 1 < nbatch else None
            U, Zb, hn_next = attention(qt, kt, vT, next_xt=xt_next)
            pending = (U, Zb, xt_cur, b)
            xt_cur = xt_next
            hn_cur = hn_next
        fuse_b[0] = pending[3]
        proj_mms(pending[0], pending[1], fuse=pending[2])

    nc.compile()
    return nc


def make_host_inputs(x, gn_scale, gn_bias, wq, bq, wk, bk, wv, bv, wo, bo,
                     n_cores=8):
    """Shard + precompute host-side arrays. Returns (in_maps, nbatch)."""
    B = x.shape[0]
    nbatch = B // n_cores
    xr = np.ascontiguousarray(np.asarray(x, np.float32).reshape(B, C, N))
    beff = (np.asarray(wo, np.float32) @ np.asarray(bv, np.float32)
            + np.asarray(bo, np.float32))
    vpack = np.zeros((C, VP), np.float32)
    vpack[:, 0] = np.asarray(gn_scale, np.float32)
    vpack[:, 1] = np.asarray(gn_bias, np.float32)
    vpack[:, 2] = np.asarray(bq, np.float32)
    vpack[:, 3] = np.asarray(bk, np.float32)
    vpack[:, 4] = beff
    cidx = np.arange(C)
    vpack[cidx, 5 + cidx // GW] = 1.0 / (GW * N)
    vpack[cidx, 21 + cidx // GW] = 1.0 / GW
    indT = np.zeros((33, C), np.float32)
    indT[cidx // GW, cidx] = np.asarray(gn_scale, np.float32)
    indT[32, :] = np.asarray(gn_bias, np.float32)
    common = {
        "wqT": np.ascontiguousarray(np.asarray(wq, np.float32).T),
        "wkT": np.ascontiguousarray(np.asarray(wk, np.float32).T),
        "wvT": np.ascontiguousarray(np.asarray(wv, np.float32).T),
        "woT": np.ascontiguousarray(np.asarray(wo, np.float32).T),
        "vpack": vpack,
        "indT": indT,
        "ones": np.ones((128, 1), np.float32),
    }
    in_maps = []
    for i in range(n_cores):
        m = dict(common)
        m["xs"] = np.ascontiguousarray(xr[i * nbatch:(i + 1) * nbatch])
        in_maps.append(m)
    return in_maps, nbatch


_NC_CACHE = {}


def _get_nc(nbatch, use_beff):
    key = (nbatch, use_beff)
    if key not in _NC_CACHE:
        _NC_CACHE[key] = build_attention_nc(nbatch=nbatch, mm_dt="f32r",
                                            n_cores=8, use_beff=use_beff)
    return _NC_CACHE[key]


def kernel(x, gn_scale, gn_bias, wq, bq, wk, bk, wv, bv, wo, bo):
    """Full-input entry point: shards over 8 NeuronCores, returns full out."""
    from concourse.bass_utils import run_bass_kernel_spmd

    x = np.asarray(x, np.float32)
    B, Cin, H, W = x.shape
    assert (Cin, H * W) == (C, N), f"unexpected shape {x.shape}"
    n_cores = 8
    assert B % n_cores == 0
    in_maps, nbatch = make_host_inputs(
        x.reshape(B, C, N), gn_scale, gn_bias, wq, bq, wk, bk, wv, bv, wo, bo,
        n_cores=n_cores)
    use_beff = bool(np.any(in_maps[0]["vpack"][:, 4]))
    nc = _get_nc(nbatch, use_beff)
    res = run_bass_kernel_spmd(nc, in_maps, core_ids=list(range(n_cores)))
    out = np.concatenate([res.results[i]["out"] for i in range(n_cores)],
                         axis=0)
    return out.reshape(B, Cin, H, W).astype(np.float32)


# revision 3
# speedup vs baseline: 1.0726x; 1.0328x over previous
"""Self-contained Trainium2 Bass kernel for nn_AttentionBlock_80315888435976.

AttentionBlock: GroupNorm(16 groups) -> 1x1-conv q/k/v -> softmax attention
over the 32x32 spatial grid -> 1x1-conv out-projection -> residual.
Input x: [32, 512, 32, 32] fp32; weights [512, 512]; all biases [512].

Distribution: data-parallel over the batch dim across 8 NeuronCores
(4 batch elements per core); weights broadcast; no collectives.
"""
import sys
sys.path.insert(0, "/opt/trn_rl_repo")

import contextlib
import numpy as np

import concourse.bass as bass
import concourse.bass_isa as bass_isa
import concourse.bacc as bacc
import concourse.tile as tile
from concourse import mybir

F32 = mybir.dt.float32
F32R = mybir.dt.float32r
U32 = mybir.dt.uint32
AF = mybir.ActivationFunctionType
OP = mybir.AluOpType

C = 512
N = 1024
G = 16
GW = C // G      # 32 channels per group
CC = C // 128    # 4 channel chunks
NM = N // 128    # 8 m chunks
NH = N // 512    # 2 free halves
EPS = 1e-6
SCALE = 1.0 / np.sqrt(C)
# vecpack columns: 0 gnsc, 1 gnb, 2 bq, 3 bk, 4 beff,
#                  5:21 indm_sums (1/(GW*N)), 21:37 indm_mv (1/GW)
VP = 37
GE = 33        # gse rows: 0..15 = groups, 32 = bias row (base-partition
               # alignment: compute-engine APs must start at multiples of 32)


def build_attention_nc(nbatch=4, mm_dt="f32r", n_cores=8, use_beff=False):
    nc = bacc.Bacc("TRN2", target_bir_lowering=False, debug=False,
                   num_devices=n_cores)
    rdt = F32R if mm_dt == "f32r" else F32

    xs = nc.dram_tensor("xs", [nbatch, C, N], F32, kind="ExternalInput")
    wq = nc.dram_tensor("wqT", [C, C], rdt, kind="ExternalInput")
    wk = nc.dram_tensor("wkT", [C, C], rdt, kind="ExternalInput")
    wv = nc.dram_tensor("wvT", [C, C], rdt, kind="ExternalInput")
    wo = nc.dram_tensor("woT", [C, C], rdt, kind="ExternalInput")
    vpack = nc.dram_tensor("vpack", [C, VP], F32, kind="ExternalInput")
    indT = nc.dram_tensor("indT", [GE, C], F32, kind="ExternalInput")
    onesd = nc.dram_tensor("ones", [128, 1], rdt, kind="ExternalInput")
    outd = nc.dram_tensor("out", [nbatch, C, N], F32, kind="ExternalOutput")

    def r(dram2d):  # [C, X] dram -> [128, CC, X] view
        return dram2d.ap().rearrange("(cc p) x -> p cc x", p=128)

    def mm(ps, lhsT, rhs, start, stop):
        nc.tensor.matmul(ps, lhsT, rhs, start=start, stop=stop)

    with tile.TileContext(nc) as tc, contextlib.ExitStack() as ctx:
        wpool = ctx.enter_context(tc.tile_pool(name="w", bufs=1))
        vecs = ctx.enter_context(tc.tile_pool(name="vecs", bufs=1))
        xpool = ctx.enter_context(tc.tile_pool(name="x", bufs=3))
        hpool = ctx.enter_context(tc.tile_pool(name="hn", bufs=1))
        qkpool = ctx.enter_context(tc.tile_pool(name="qk", bufs=1))
        vpool = ctx.enter_context(tc.tile_pool(name="v", bufs=1))
        epool = ctx.enter_context(tc.tile_pool(name="e", bufs=3))
        upool = ctx.enter_context(tc.tile_pool(name="u", bufs=1))
        opool = ctx.enter_context(tc.tile_pool(name="o", bufs=1))
        rpool = ctx.enter_context(tc.tile_pool(name="r", bufs=2))
        stats = ctx.enter_context(tc.tile_pool(name="st", bufs=2))
        ps_pool = ctx.enter_context(tc.tile_pool(name="ps", bufs=4, space="PSUM"))
        acc_pool = ctx.enter_context(tc.tile_pool(name="acc", bufs=1, space="PSUM"))

        # ---- constants (3 DMAs; HWDGE issue pipe costs ~0.65us per DMA) ----
        vp_sb = vecs.tile([128, CC, VP], F32, tag="vp")
        indT_sb = vecs.tile([GE, CC, 128], F32, tag="indT")
        ones_sb = vecs.tile([128, 1], rdt, tag="ones")
        gse = vecs.tile([GE, 2], F32, tag="gse")
        eps_sb = vecs.tile([G, 1], F32, tag="eps")
        magic_sb = vecs.tile([G, 1], U32, tag="magic")
        c15_sb = vecs.tile([G, 1], F32, tag="c15")
        nc.sync.dma_start(out=vp_sb[:], in_=r(vpack))
        nc.vector.memset(eps_sb[:], EPS)
        nc.vector.memset(magic_sb[:], 0x5f3759df)
        nc.vector.memset(c15_sb[:], 1.5)
        nc.vector.memset(gse[32:GE, 0:1], 0.0)
        nc.vector.memset(gse[32:GE, 1:2], 1.0)
        gnsc_sb = vp_sb[:, :, 0:1]
        gnb_sb = vp_sb[:, :, 1:2]
        bq_sb = vp_sb[:, :, 2:3]
        bk_sb = vp_sb[:, :, 3:4]
        beff_sb = vp_sb[:, :, 4:5]

        def stat_op(xt, sums, scr, k):
            """k-th of 8 ACT ops accumulating per-channel sum / sum-sq.
            scr is a scratch dummy output (only accum_out matters)."""
            cc, which = divmod(k, 2)
            nc.scalar.activation(out=scr[:, cc, :],
                                 in_=xt[:, cc, :],
                                 func=(AF.Copy if which == 0 else AF.Square),
                                 accum_out=sums[:, cc, which:which + 1])

        def gn_stat_tiles():
            # (scr is not allocated here: the next batch's hn tile doubles as
            # the dummy activation output until its real write in h1)
            return stats.tile([128, CC, 2], F32, tag="sums", name="sums")

        def gn_sum_mms(sums, dve_chunks=()):
            ps_g = ps_pool.tile([G, 2], F32, tag="ps")
            for cc in range(CC):
                col = slice(21, 37) if cc in dve_chunks else slice(5, 21)
                nc.tensor.matmul(ps_g[:], vp_sb[:, cc, col], sums[:, cc, :],
                                 start=(cc == 0), stop=(cc == CC - 1))
            return ps_g

        # ---- batch-0 x load: per chunk, stats split ACT/DVE ----
        xt0 = xpool.tile([128, CC, N], F32, tag="x", name="xt0")
        hn0 = hpool.tile([128, CC, N], F32R if mm_dt == "f32r" else F32,
                         tag="hn", name="hn0")
        sums0 = gn_stat_tiles()
        st6_0 = stats.tile([128, CC, 2, 6], F32, tag="st6")
        mv0 = stats.tile([128, CC, 2], F32, tag="mv")
        b0_dve_chunks = (1, 3)
        for cc in range(CC):
            nc.sync.dma_start(out=xt0[:, cc, :],
                              in_=xs.ap()[0][bass.ts(cc, 128), :])
            if cc in b0_dve_chunks:
                # DVE path -> sums0[:, cc] = [mu_c, mu_c^2 + var_c]
                for h in range(2):
                    nc.vector.bn_stats(out=st6_0[:, cc, h, :],
                                       in_=xt0[:, cc, bass.ts(h, 512)])
                nc.vector.bn_aggr(out=mv0[:, cc, :], in_=st6_0[:, cc, :, :])
                nc.vector.tensor_mul(out=sums0[:, cc, 1:2],
                                     in0=mv0[:, cc, 0:1], in1=mv0[:, cc, 0:1])
                nc.vector.tensor_add(out=sums0[:, cc, 1:2],
                                     in0=sums0[:, cc, 1:2], in1=mv0[:, cc, 1:2])
                nc.vector.tensor_copy(out=sums0[:, cc, 0:1],
                                      in_=mv0[:, cc, 0:1])
            else:
                stat_op(xt0, sums0, hn0, 2 * cc)
                stat_op(xt0, sums0, hn0, 2 * cc + 1)

        nc.sync.dma_start(
            out=indT_sb[:], in_=indT.ap().rearrange("g (cc p) -> g cc p", p=128))
        nc.sync.dma_start(out=ones_sb[:], in_=onesd.ap())

        wq_sb = wpool.tile([128, CC, C], rdt, tag="wq")
        wk_sb = wpool.tile([128, CC, C], rdt, tag="wk")
        wv_sb = wpool.tile([128, CC, C], rdt, tag="wv")
        wo_sb = wpool.tile([128, CC, C], rdt, tag="wo")
        nc.sync.dma_start(out=wq_sb[:], in_=r(wq))
        nc.sync.dma_start(out=wk_sb[:], in_=r(wk))
        nc.sync.dma_start(out=wv_sb[:], in_=r(wv))
        nc.sync.dma_start(out=wo_sb[:], in_=r(wo))

        def load_x(b):
            xt = xpool.tile([128, CC, N], F32, tag="x")
            nc.sync.dma_start(
                out=xt[:], in_=xs.ap()[b].rearrange("(cc p) n -> p cc n", p=128))
            return xt

        def gn_finish(ps_g):
            """[mu_g, m2_g] -> gse rows 0..15 = [rstd_g, -mu_g*rstd_g]."""
            gsb = stats.tile([G, 2], F32, tag="gsb")
            varg = stats.tile([G, 1], F32, tag="varg")
            nc.vector.tensor_copy(out=gsb[:], in_=ps_g[:])
            nc.vector.tensor_mul(out=varg[:], in0=gsb[:, 0:1], in1=gsb[:, 0:1])
            nc.vector.tensor_tensor(out=varg[:], in0=gsb[:, 1:2], in1=varg[:],
                                    op=OP.subtract)
            nc.vector.tensor_scalar_add(out=varg[:], in0=varg[:], scalar1=EPS)
            y = stats.tile([G, 1], F32, tag="nwt_y")
            vh = stats.tile([G, 1], F32, tag="nwt_vh")
            t = stats.tile([G, 1], F32, tag="nwt_t")
            nc.vector.tensor_scalar(out=t[:].bitcast(U32),
                                    in0=varg[:].bitcast(U32),
                                    scalar1=1, scalar2=None,
                                    op0=OP.logical_shift_right)
            nc.vector.tensor_tensor(out=y[:].bitcast(U32), in0=magic_sb[:],
                                    in1=t[:].bitcast(U32), op=OP.subtract)
            nc.vector.tensor_scalar_mul(out=vh[:], in0=varg[:], scalar1=0.5)
            for it in range(2):
                nc.vector.tensor_mul(out=t[:], in0=y[:], in1=y[:])
                nc.vector.tensor_mul(out=t[:], in0=vh[:], in1=t[:])
                nc.vector.tensor_tensor(out=t[:], in0=c15_sb[:], in1=t[:],
                                        op=OP.subtract)
                dst = gse[0:G, 0:1] if it == 1 else y[:]
                nc.vector.tensor_mul(out=dst, in0=y[:], in1=t[:])
            nc.vector.tensor_mul(out=t[:], in0=gsb[:, 0:1], in1=gse[0:G, 0:1])
            nc.vector.tensor_scalar_mul(out=gse[0:G, 1:2], in0=t[:],
                                        scalar1=-1.0)

        def gn_ab(dve=False):
            ab_sb = stats.tile([128, CC, 2], F32, tag="ab_sb")
            for cc in range(CC):
                ps_cb = ps_pool.tile([128, 2], F32, tag="ps")
                nc.tensor.matmul(ps_cb[:], indT_sb[:, cc, :], gse[:],
                                 start=True, stop=True)
                if dve:
                    nc.vector.tensor_copy(out=ab_sb[:, cc, :], in_=ps_cb[:])
                else:
                    nc.scalar.activation(out=ab_sb[:, cc, :], in_=ps_cb[:],
                                         func=AF.Copy)
            return ab_sb

        def gn_hn_apply(xt, ab_sb, hn, cc, dve_extra=False):
            if cc % 2 or (dve_extra and cc == 2):
                nc.vector.tensor_scalar(out=hn[:, cc, :], in0=xt[:, cc, :],
                                        scalar1=ab_sb[:, cc, 0:1],
                                        scalar2=ab_sb[:, cc, 1:2],
                                        op0=OP.mult, op1=OP.add)
            else:
                nc.scalar.activation(out=hn[:, cc, :], in_=xt[:, cc, :],
                                     func=AF.Identity,
                                     scale=ab_sb[:, cc, 0:1],
                                     bias=ab_sb[:, cc, 1:2])

        def gn_apply(xt, hn):
            ab_sb = gn_ab(dve=True)
            for cc in range(CC):
                gn_hn_apply(xt, ab_sb, hn, cc, dve_extra=True)
            return hn

        def qkv(hn):
            qt = qkpool.tile([128, CC, N], rdt, tag="q")
            kt = qkpool.tile([128, CC, N], rdt, tag="k")
            for wsb, bias_sb, dst in ((wq_sb, bq_sb, qt), (wk_sb, bk_sb, kt)):
                for co in range(CC):
                    for h in range(NH):
                        ps_t = ps_pool.tile([128, 512], F32, tag="ps")
                        for ci in range(CC):
                            mm(ps_t[:], wsb[:, ci, bass.ts(co, 128)],
                               hn[:, ci, bass.ts(h, 512)], ci == 0, ci == CC - 1)
                        nc.vector.tensor_scalar_add(
                            out=dst[:, co, bass.ts(h, 512)], in0=ps_t[:],
                            scalar1=bias_sb[:, co, :])
            vT = vpool.tile([128, NM, C], rdt, tag="vT")
            for mo in range(NM):
                ps_t = ps_pool.tile([128, 512], F32, tag="ps")
                for ci in range(CC):
                    mm(ps_t[:], hn[:, ci, bass.ts(mo, 128)], wv_sb[:, ci, :],
                       ci == 0, ci == CC - 1)
                nc.vector.tensor_copy(out=vT[:, mo, :], in_=ps_t[:])
            return qt, kt, vT

        def attention(qt, kt, vT, next_xt=None):
            Zb = rpool.tile([128, N], F32, tag="Zb")
            U = upool.tile([128, CC, N], rdt, tag="U")
            sums_next = hn_next = None
            ab_next = [None]
            if next_xt is not None:
                sums_next = gn_stat_tiles()
                hn_next = hpool.tile([128, CC, N], rdt, tag="hn", name="hn")
            def emit_scores(h, mo):
                ps_s = ps_pool.tile([128, 512], F32, tag="ps", name="ps_s")
                for ci in range(CC):
                    mm(ps_s[:], kt[:, ci, bass.ts(mo, 128)],
                       qt[:, ci, bass.ts(h, 512)], ci == 0, ci == CC - 1)
                ech = epool.tile([128, 512], rdt, tag="e", name="ech")
                nc.scalar.activation(out=ech[:], in_=ps_s[:], func=AF.Exp,
                                     scale=SCALE)
                return ech

            ech = emit_scores(0, 0)
            for h in range(NH):
                # Z[n] = sum_m E[m,n] on GpSimd (idle otherwise): per-chunk
                # partition_all_reduce, accumulated across chunks on DVE.
                zacc = rpool.tile([128, 512], F32, tag="zacc", name="zacc")
                ps_u = acc_pool.tile([128, CC, 512], F32, tag="acc")
                for mo in range(NM):
                    # one-group score lookahead keeps PE busy across the
                    # exp latency and the half-boundary U-psum handoff
                    ech_next = None
                    if mo + 1 < NM:
                        ech_next = emit_scores(h, mo + 1)
                    elif h + 1 < NH:
                        ech_next = emit_scores(h + 1, 0)
                    if mo == 0:
                        nc.gpsimd.partition_all_reduce(
                            zacc[:], ech[:].bitcast(F32), channels=128,
                            reduce_op=bass_isa.ReduceOp.add)
                    elif mo < NM - 1:
                        zp = rpool.tile([128, 512], F32, tag="zp", name="zp")
                        nc.gpsimd.partition_all_reduce(
                            zp[:], ech[:].bitcast(F32), channels=128,
                            reduce_op=bass_isa.ReduceOp.add)
                        nc.vector.tensor_add(out=zacc[:], in0=zacc[:],
                                             in1=zp[:])
                    else:
                        ech_last = ech  # z-accumulate deferred past U copies
                    for co in range(CC):
                        mm(ps_u[:, co, :], vT[:, mo, bass.ts(co, 128)], ech[:],
                           mo == 0, mo == NM - 1)
                    ech = ech_next
                    if h == 0 and next_xt is not None:
                        stat_op(next_xt, sums_next, hn_next, mo)
                    if h == 1 and next_xt is not None:
                        if mo == 0:
                            ps_g = gn_sum_mms(sums_next)
                            gn_finish(ps_g)
                        elif mo == 2:
                            ab_next[0] = gn_ab()
                        elif mo >= 4:
                            gn_hn_apply(next_xt, ab_next[0], hn_next, mo - 4)
                sl = bass.ts(h, 512)
                if h == 0:
                    # U copies first: they gate the next half's U accumulation
                    for co in range(CC):
                        nc.scalar.activation(out=U[:, co, sl],
                                             in_=ps_u[:, co, :], func=AF.Copy)
                else:
                    for co in range(CC):
                        nc.scalar.activation(out=U[:, co, sl],
                                             in_=ps_u[:, co, :], func=AF.Copy)
                zp = rpool.tile([128, 512], F32, tag="zp", name="zp")
                nc.gpsimd.partition_all_reduce(
                    zp[:], ech_last[:].bitcast(F32), channels=128,
                    reduce_op=bass_isa.ReduceOp.add)
                nc.vector.tensor_add(out=zacc[:], in0=zacc[:], in1=zp[:])
                nc.vector.reciprocal(out=Zb[:, sl], in_=zacc[:])
            return U, Zb, hn_next

        def proj_mms(U, Zb, fuse=None):
            out_sb = opool.tile([128, CC, N], F32, tag="out")
            for h in range(NH):
                sl = bass.ts(h, 512)
                for co in range(CC):
                    ps_o = ps_pool.tile([128, 512], F32, tag="ps")
                    for ci in range(CC):
                        mm(ps_o[:], wo_sb[:, ci, bass.ts(co, 128)],
                           U[:, ci, sl], ci == 0, ci == CC - 1)
                    nc.vector.tensor_mul(out=out_sb[:, co, sl],
                                         in0=ps_o[:], in1=Zb[:, sl])
                    if fuse is not None:
                        nc.vector.tensor_add(out=out_sb[:, co, sl],
                                             in0=out_sb[:, co, sl],
                                             in1=fuse[:, co, sl])
                        if use_beff:
                            nc.vector.tensor_scalar_add(
                                out=out_sb[:, co, sl],
                                in0=out_sb[:, co, sl],
                                scalar1=beff_sb[:, co, :])
                if fuse is not None:
                    nc.gpsimd.dma_start(
                        out=outd.ap()[fuse_b[0]].rearrange(
                            "(cc p) n -> p cc n", p=128)[:, :, sl],
                        in_=out_sb[:, :, sl])
            return out_sb

        def epilogue(out_sb, xt, b):
            for h in range(NH):
                sl = bass.ts(h, 512)
                for co in range(CC):
                    nc.vector.tensor_add(out=out_sb[:, co, sl],
                                         in0=out_sb[:, co, sl],
                                         in1=xt[:, co, sl])
                    if use_beff:
                        nc.vector.tensor_scalar_add(out=out_sb[:, co, sl],
                                                    in0=out_sb[:, co, sl],
                                                    scalar1=beff_sb[:, co, :])
                nc.gpsimd.dma_start(
                    out=outd.ap()[b].rearrange("(cc p) n -> p cc n",
                                               p=128)[:, :, sl],
                    in_=out_sb[:, :, sl])

        # ---- software-pipelined batch loop ----
        # GN of batch b+1 (stats, group matmuls, Newton rsqrt, broadcast,
        # affine apply) is emitted INSIDE attention(b), where PE/ACT/DVE
        # all have slack; batch boundaries carry only proj -> qkv.
        pending = None
        xt_cur = xt0
        hn_cur = None
        fuse_b = [None]
        for b in range(nbatch):
            if b == 0:
                ps_g = gn_sum_mms(sums0, dve_chunks=b0_dve_chunks)
                gn_finish(ps_g)
                hn_cur = gn_apply(xt_cur, hn0)
            out_prev = None
            if pending is not None:
                out_prev = proj_mms(pending[0], pending[1])
            qt, kt, vT = qkv(hn_cur)
            if pending is not None:
                epilogue(out_prev, pending[2], pending[3])
            xt_next = load_x(b + 1) if b + 1 < nbatch else None
            U, Zb, hn_next = attention(qt, kt, vT, next_xt=xt_next)
            pending = (U, Zb, xt_cur, b)
            xt_cur = xt_next
            hn_cur = hn_next
        fuse_b[0] = pending[3]
        proj_mms(pending[0], pending[1], fuse=pending[2])

    nc.compile()
    return nc


def make_host_inputs(x, gn_scale, gn_bias, wq, bq, wk, bk, wv, bv, wo, bo,
                     n_cores=8):
    """Shard + precompute host-side arrays. Returns (in_maps, nbatch)."""
    B = x.shape[0]
    nbatch = B // n_cores
    xr = np.ascontiguousarray(np.asarray(x, np.float32).reshape(B, C, N))
    beff = (np.asarray(wo, np.float32) @ np.asarray(bv, np.float32)
            + np.asarray(bo, np.float32))
    vpack = np.zeros((C, VP), np.float32)
    vpack[:, 0] = np.asarray(gn_scale, np.float32)
    vpack[:, 1] = np.asarray(gn_bias, np.float32)
    vpack[:, 2] = np.asarray(bq, np.float32)
    vpack[:, 3] = np.asarray(bk, np.float32)
    vpack[:, 4] = beff
    cidx = np.arange(C)
    vpack[cidx, 5 + cidx // GW] = 1.0 / (GW * N)
    vpack[cidx, 21 + cidx // GW] = 1.0 / GW
    indT = np.zeros((33, C), np.float32)
    indT[cidx // GW, cidx] = np.asarray(gn_scale, np.float32)
    indT[32, :] = np.asarray(gn_bias, np.float32)
    common = {
        "wqT": np.ascontiguousarray(np.asarray(wq, np.float32).T),
        "wkT": np.ascontiguousarray(np.asarray(wk, np.float32).T),
        "wvT": np.ascontiguousarray(np.asarray(wv, np.float32).T),
        "woT": np.ascontiguousarray(np.asarray(wo, np.float32).T),
        "vpack": vpack,
        "indT": indT,
        "ones": np.ones((128, 1), np.float32),
    }
    in_maps = []
    for i in range(n_cores):
        m = dict(common)
        m["xs"] = np.ascontiguousarray(xr[i * nbatch:(i + 1) * nbatch])
        in_maps.append(m)
    return in_maps, nbatch


_NC_CACHE = {}


def _get_nc(nbatch, use_beff):
    key = (nbatch, use_beff)
    if key not in _NC_CACHE:
        _NC_CACHE[key] = build_attention_nc(nbatch=nbatch, mm_dt="f32r",
                                            n_cores=8, use_beff=use_beff)
    return _NC_CACHE[key]


def kernel(x, gn_scale, gn_bias, wq, bq, wk, bk, wv, bv, wo, bo):
    """Full-input entry point: shards over 8 NeuronCores, returns full out."""
    from concourse.bass_utils import run_bass_kernel_spmd

    x = np.asarray(x, np.float32)
    B, Cin, H, W = x.shape
    assert (Cin, H * W) == (C, N), f"unexpected shape {x.shape}"
    n_cores = 8
    assert B % n_cores == 0
    in_maps, nbatch = make_host_inputs(
        x.reshape(B, C, N), gn_scale, gn_bias, wq, bq, wk, bk, wv, bv, wo, bo,
        n_cores=n_cores)
    use_beff = bool(np.any(in_maps[0]["vpack"][:, 4]))
    nc = _get_nc(nbatch, use_beff)
    res = run_bass_kernel_spmd(nc, in_maps, core_ids=list(range(n_cores)))
    out = np.concatenate([res.results[i]["out"] for i in range(n_cores)],
                         axis=0)
    return out.reshape(B, Cin, H, W).astype(np.float32)


# revision 4
# speedup vs baseline: 1.3264x; 1.2367x over previous
"""Self-contained Trainium2 Bass kernel for nn_AttentionBlock_80315888435976.

AttentionBlock: GroupNorm(16 groups) -> 1x1-conv q/k/v -> softmax attention
over the 32x32 spatial grid -> 1x1-conv out-projection -> residual.
Input x: [32, 512, 32, 32] fp32; weights [512, 512]; all biases [512].

Distribution: data-parallel over the batch dim across 8 NeuronCores
(4 batch elements per core); weights broadcast; no collectives.
"""
import sys
sys.path.insert(0, "/opt/trn_rl_repo")

import contextlib
import numpy as np

import concourse.bass as bass
import concourse.bass_isa as bass_isa
import concourse.bacc as bacc
import concourse.tile as tile
from concourse import mybir

F32 = mybir.dt.float32
F32R = mybir.dt.float32r
U32 = mybir.dt.uint32
AF = mybir.ActivationFunctionType
OP = mybir.AluOpType

C = 512
N = 1024
G = 16
GW = C // G      # 32 channels per group
CC = C // 128    # 4 channel chunks
NM = N // 128    # 8 m chunks
NH = N // 512    # 2 free halves
EPS = 1e-6
SCALE = 1.0 / np.sqrt(C)
# vecpack columns: 0 gnsc, 1 gnb, 2 bq, 3 bk, 4 beff,
#                  5:21 indm_sums (1/(GW*N)), 21:37 indm_mv (1/GW)
VP = 37
GE = 33        # gse rows: 0..15 = groups, 32 = bias row (base-partition
               # alignment: compute-engine APs must start at multiples of 32)


def build_attention_nc(nbatch=4, mm_dt="f32r", n_cores=8, use_beff=False):
    nc = bacc.Bacc("TRN2", target_bir_lowering=False, debug=False,
                   num_devices=n_cores)
    rdt = F32R if mm_dt == "f32r" else F32

    xs = nc.dram_tensor("xs", [nbatch, C, N], F32, kind="ExternalInput")
    wq = nc.dram_tensor("wqT", [C, C], rdt, kind="ExternalInput")
    wk = nc.dram_tensor("wkT", [C, C], rdt, kind="ExternalInput")
    wv = nc.dram_tensor("wvT", [C, C], rdt, kind="ExternalInput")
    wo = nc.dram_tensor("woT", [C, C], rdt, kind="ExternalInput")
    vpack = nc.dram_tensor("vpack", [C, VP], F32, kind="ExternalInput")
    indT = nc.dram_tensor("indT", [GE, C], F32, kind="ExternalInput")
    onesd = nc.dram_tensor("ones", [128, 1], rdt, kind="ExternalInput")
    outd = nc.dram_tensor("out", [nbatch, C, N], F32, kind="ExternalOutput")

    def r(dram2d):  # [C, X] dram -> [128, CC, X] view
        return dram2d.ap().rearrange("(cc p) x -> p cc x", p=128)

    def mm(ps, lhsT, rhs, start, stop):
        nc.tensor.matmul(ps, lhsT, rhs, start=start, stop=stop)

    with tile.TileContext(nc) as tc, contextlib.ExitStack() as ctx:
        wpool = ctx.enter_context(tc.tile_pool(name="w", bufs=1))
        vecs = ctx.enter_context(tc.tile_pool(name="vecs", bufs=1))
        xpool = ctx.enter_context(tc.tile_pool(name="x", bufs=3))
        hpool = ctx.enter_context(tc.tile_pool(name="hn", bufs=1))
        qkpool = ctx.enter_context(tc.tile_pool(name="qk", bufs=1))
        vpool = ctx.enter_context(tc.tile_pool(name="v", bufs=1))
        epool = ctx.enter_context(tc.tile_pool(name="e", bufs=3))
        upool = ctx.enter_context(tc.tile_pool(name="u", bufs=1))
        opool = ctx.enter_context(tc.tile_pool(name="o", bufs=1))
        rpool = ctx.enter_context(tc.tile_pool(name="r", bufs=2))
        stats = ctx.enter_context(tc.tile_pool(name="st", bufs=2))
        ps_pool = ctx.enter_context(tc.tile_pool(name="ps", bufs=4, space="PSUM"))
        acc_pool = ctx.enter_context(tc.tile_pool(name="acc", bufs=1, space="PSUM"))

        # ---- constants (3 DMAs; HWDGE issue pipe costs ~0.65us per DMA) ----
        vp_sb = vecs.tile([128, CC, VP], F32, tag="vp")
        indT_sb = vecs.tile([GE, CC, 128], F32, tag="indT")
        ones_sb = vecs.tile([128, 1], rdt, tag="ones")
        gse = vecs.tile([GE, 2], F32, tag="gse")
        eps_sb = vecs.tile([G, 1], F32, tag="eps")
        magic_sb = vecs.tile([G, 1], U32, tag="magic")
        c15_sb = vecs.tile([G, 1], F32, tag="c15")
        nc.vector.memset(eps_sb[:], EPS)
        nc.vector.memset(magic_sb[:], 0x5f3759df)
        nc.vector.memset(c15_sb[:], 1.5)
        nc.vector.memset(gse[32:GE, 0:1], 0.0)
        nc.vector.memset(gse[32:GE, 1:2], 1.0)
        gnsc_sb = vp_sb[:, :, 0:1]
        gnb_sb = vp_sb[:, :, 1:2]
        bq_sb = vp_sb[:, :, 2:3]
        bk_sb = vp_sb[:, :, 3:4]
        beff_sb = vp_sb[:, :, 4:5]

        def stat_op(xt, sums, scr, k):
            """k-th of 8 ACT ops accumulating per-channel sum / sum-sq.
            scr is a scratch dummy output (only accum_out matters)."""
            cc, which = divmod(k, 2)
            nc.scalar.activation(out=scr[:, cc, :],
                                 in_=xt[:, cc, :],
                                 func=(AF.Copy if which == 0 else AF.Square),
                                 accum_out=sums[:, cc, which:which + 1])

        def gn_stat_tiles():
            # (scr is not allocated here: the next batch's hn tile doubles as
            # the dummy activation output until its real write in h1)
            return stats.tile([128, CC, 2], F32, tag="sums", name="sums")

        def gn_sum_mms(sums, dve_chunks=()):
            ps_g = ps_pool.tile([G, 2], F32, tag="ps")
            for cc in range(CC):
                col = slice(21, 37) if cc in dve_chunks else slice(5, 21)
                nc.tensor.matmul(ps_g[:], vp_sb[:, cc, col], sums[:, cc, :],
                                 start=(cc == 0), stop=(cc == CC - 1))
            return ps_g

        # ---- batch-0 x load: per chunk, stats split ACT/DVE ----
        xt0 = xpool.tile([128, CC, N], F32, tag="x", name="xt0")
        hn0 = hpool.tile([128, CC, N], F32R if mm_dt == "f32r" else F32,
                         tag="hn", name="hn0")
        sums0 = gn_stat_tiles()
        st6_0 = stats.tile([128, CC, 2, 6], F32, tag="st6")
        mv0 = stats.tile([128, CC, 2], F32, tag="mv")
        b0_dve_chunks = (1, 3)
        for cc in range(CC):
            nc.sync.dma_start(out=xt0[:, cc, :],
                              in_=xs.ap()[0][bass.ts(cc, 128), :])
            if cc in b0_dve_chunks:
                # DVE path -> sums0[:, cc] = [mu_c, mu_c^2 + var_c]
                for h in range(2):
                    nc.vector.bn_stats(out=st6_0[:, cc, h, :],
                                       in_=xt0[:, cc, bass.ts(h, 512)])
                nc.vector.bn_aggr(out=mv0[:, cc, :], in_=st6_0[:, cc, :, :])
                nc.vector.tensor_mul(out=sums0[:, cc, 1:2],
                                     in0=mv0[:, cc, 0:1], in1=mv0[:, cc, 0:1])
                nc.vector.tensor_add(out=sums0[:, cc, 1:2],
                                     in0=sums0[:, cc, 1:2], in1=mv0[:, cc, 1:2])
                nc.vector.tensor_copy(out=sums0[:, cc, 0:1],
                                      in_=mv0[:, cc, 0:1])
            else:
                stat_op(xt0, sums0, hn0, 2 * cc)
                stat_op(xt0, sums0, hn0, 2 * cc + 1)

        nc.sync.dma_start(out=vp_sb[:], in_=r(vpack))
        nc.sync.dma_start(
            out=indT_sb[:], in_=indT.ap().rearrange("g (cc p) -> g cc p", p=128))
        nc.sync.dma_start(out=ones_sb[:], in_=onesd.ap())

        wq_sb = wpool.tile([128, CC, C], rdt, tag="wq")
        wk_sb = wpool.tile([128, CC, C], rdt, tag="wk")
        wv_sb = wpool.tile([128, CC, C], rdt, tag="wv")
        wo_sb = wpool.tile([128, CC, C], rdt, tag="wo")
        nc.sync.dma_start(out=wq_sb[:], in_=r(wq))
        nc.sync.dma_start(out=wk_sb[:], in_=r(wk))
        nc.sync.dma_start(out=wv_sb[:], in_=r(wv))
        nc.sync.dma_start(out=wo_sb[:], in_=r(wo))

        def load_x(b):
            xt = xpool.tile([128, CC, N], F32, tag="x")
            nc.sync.dma_start(
                out=xt[:], in_=xs.ap()[b].rearrange("(cc p) n -> p cc n", p=128))
            return xt

        def gn_finish(ps_g):
            """[mu_g, m2_g] -> gse rows 0..15 = [rstd_g, -mu_g*rstd_g]."""
            gsb = stats.tile([G, 2], F32, tag="gsb")
            varg = stats.tile([G, 1], F32, tag="varg")
            nc.vector.tensor_copy(out=gsb[:], in_=ps_g[:])
            nc.vector.tensor_mul(out=varg[:], in0=gsb[:, 0:1], in1=gsb[:, 0:1])
            nc.vector.tensor_tensor(out=varg[:], in0=gsb[:, 1:2], in1=varg[:],
                                    op=OP.subtract)
            nc.vector.tensor_scalar_add(out=varg[:], in0=varg[:], scalar1=EPS)
            y = stats.tile([G, 1], F32, tag="nwt_y")
            vh = stats.tile([G, 1], F32, tag="nwt_vh")
            t = stats.tile([G, 1], F32, tag="nwt_t")
            nc.vector.tensor_scalar(out=t[:].bitcast(U32),
                                    in0=varg[:].bitcast(U32),
                                    scalar1=1, scalar2=None,
                                    op0=OP.logical_shift_right)
            nc.vector.tensor_tensor(out=y[:].bitcast(U32), in0=magic_sb[:],
                                    in1=t[:].bitcast(U32), op=OP.subtract)
            nc.vector.tensor_scalar_mul(out=vh[:], in0=varg[:], scalar1=0.5)
            for it in range(2):
                nc.vector.tensor_mul(out=t[:], in0=y[:], in1=y[:])
                nc.vector.tensor_mul(out=t[:], in0=vh[:], in1=t[:])
                nc.vector.tensor_tensor(out=t[:], in0=c15_sb[:], in1=t[:],
                                        op=OP.subtract)
                dst = gse[0:G, 0:1] if it == 1 else y[:]
                nc.vector.tensor_mul(out=dst, in0=y[:], in1=t[:])
            nc.vector.tensor_mul(out=t[:], in0=gsb[:, 0:1], in1=gse[0:G, 0:1])
            nc.vector.tensor_scalar_mul(out=gse[0:G, 1:2], in0=t[:],
                                        scalar1=-1.0)

        def gn_ab(dve=False):
            ab_sb = stats.tile([128, CC, 2], F32, tag="ab_sb")
            for cc in range(CC):
                ps_cb = ps_pool.tile([128, 2], F32, tag="ps")
                nc.tensor.matmul(ps_cb[:], indT_sb[:, cc, :], gse[:],
                                 start=True, stop=True)
                if dve:
                    nc.vector.tensor_copy(out=ab_sb[:, cc, :], in_=ps_cb[:])
                else:
                    nc.scalar.activation(out=ab_sb[:, cc, :], in_=ps_cb[:],
                                         func=AF.Copy)
            return ab_sb

        def gn_hn_apply(xt, ab_sb, hn, cc, dve_extra=False):
            if cc % 2 or (dve_extra and cc == 2):
                nc.vector.tensor_scalar(out=hn[:, cc, :], in0=xt[:, cc, :],
                                        scalar1=ab_sb[:, cc, 0:1],
                                        scalar2=ab_sb[:, cc, 1:2],
                                        op0=OP.mult, op1=OP.add)
            else:
                nc.scalar.activation(out=hn[:, cc, :], in_=xt[:, cc, :],
                                     func=AF.Identity,
                                     scale=ab_sb[:, cc, 0:1],
                                     bias=ab_sb[:, cc, 1:2])

        def gn_apply(xt, hn):
            ab_sb = gn_ab(dve=True)
            for cc in range(CC):
                gn_hn_apply(xt, ab_sb, hn, cc, dve_extra=True)
            return hn

        def qkv(hn):
            qt = qkpool.tile([128, CC, N], rdt, tag="q")
            kt = qkpool.tile([128, CC, N], rdt, tag="k")
            for wsb, bias_sb, dst in ((wq_sb, bq_sb, qt), (wk_sb, bk_sb, kt)):
                for co in range(CC):
                    for h in range(NH):
                        ps_t = ps_pool.tile([128, 512], F32, tag="ps")
                        for ci in range(CC):
                            mm(ps_t[:], wsb[:, ci, bass.ts(co, 128)],
                               hn[:, ci, bass.ts(h, 512)], ci == 0, ci == CC - 1)
                        nc.vector.tensor_scalar_add(
                            out=dst[:, co, bass.ts(h, 512)], in0=ps_t[:],
                            scalar1=bias_sb[:, co, :])
            vT = vpool.tile([128, NM, C], rdt, tag="vT")
            for mo in range(NM):
                ps_t = ps_pool.tile([128, 512], F32, tag="ps")
                for ci in range(CC):
                    mm(ps_t[:], hn[:, ci, bass.ts(mo, 128)], wv_sb[:, ci, :],
                       ci == 0, ci == CC - 1)
                nc.vector.tensor_copy(out=vT[:, mo, :], in_=ps_t[:])
            return qt, kt, vT

        def attention(qt, kt, vT, next_xt=None):
            Zb = rpool.tile([128, N], F32, tag="Zb")
            U = upool.tile([128, CC, N], rdt, tag="U")
            sums_next = hn_next = None
            ab_next = [None]
            if next_xt is not None:
                sums_next = gn_stat_tiles()
                hn_next = hpool.tile([128, CC, N], rdt, tag="hn", name="hn")

            def emit_scores(h, mo):
                ps_s = ps_pool.tile([128, 512], F32, tag="ps", name="ps_s")
                for ci in range(CC):
                    mm(ps_s[:], kt[:, ci, bass.ts(mo, 128)],
                       qt[:, ci, bass.ts(h, 512)], ci == 0, ci == CC - 1)
                ech = epool.tile([128, 512], rdt, tag="e", name="ech")
                nc.scalar.activation(out=ech[:], in_=ps_s[:], func=AF.Exp,
                                     scale=SCALE)
                return ech

            # score-group lookahead: normally 1 group ahead; 2 across the
            # half boundary so the deferred h0 tail copies queue behind exps
            sched = [(h, mo) for h in range(NH) for mo in range(NM)]
            emitted = {}
            ptr = [0]

            def ensure(upto):
                while ptr[0] < len(sched) and ptr[0] <= upto:
                    hh, mm_ = sched[ptr[0]]
                    emitted[(hh, mm_)] = emit_scores(hh, mm_)
                    ptr[0] += 1

            def tail(h, ps_ua, ps_ub, zacc, ech_last):
                sl = bass.ts(h, 512)
                for co in range(CC):
                    pu = ps_ua if co < 2 else ps_ub
                    nc.scalar.activation(out=U[:, co, sl],
                                         in_=pu[:, co % 2, :], func=AF.Copy)
                zp = rpool.tile([128, 512], F32, tag="zp", name="zp")
                nc.gpsimd.partition_all_reduce(
                    zp[:], ech_last[:].bitcast(F32), channels=128,
                    reduce_op=bass_isa.ReduceOp.add)
                nc.vector.tensor_add(out=zacc[:], in0=zacc[:], in1=zp[:])
                nc.vector.reciprocal(out=Zb[:, sl], in_=zacc[:])

            ensure(1)
            for h in range(NH):
                zacc = rpool.tile([128, 512], F32, tag="zacc", name="zacc")
                ps_ua = ps_ub = None
                ech_last = None
                for mo in range(NM):
                    i = h * NM + mo
                    ensure(i + 1)
                    ech = emitted.pop((h, mo))
                    if mo == 0:
                        ps_ua = acc_pool.tile([128, 2, 512], F32, tag="acca",
                                              name="ps_ua")
                        ps_ub = acc_pool.tile([128, 2, 512], F32, tag="accb",
                                              name="ps_ub")
                        nc.gpsimd.partition_all_reduce(
                            zacc[:], ech[:].bitcast(F32), channels=128,
                            reduce_op=bass_isa.ReduceOp.add)
                    elif mo < NM - 1:
                        zp = rpool.tile([128, 512], F32, tag="zp", name="zp")
                        nc.gpsimd.partition_all_reduce(
                            zp[:], ech[:].bitcast(F32), channels=128,
                            reduce_op=bass_isa.ReduceOp.add)
                        nc.vector.tensor_add(out=zacc[:], in0=zacc[:],
                                             in1=zp[:])
                    else:
                        ech_last = ech  # z-accumulate deferred past U copies
                    for co in range(CC):
                        pu = ps_ua if co < 2 else ps_ub
                        mm(pu[:, co % 2, :], vT[:, mo, bass.ts(co, 128)],
                           ech[:], mo == 0, mo == NM - 1)
                    if h == 0 and next_xt is not None:
                        stat_op(next_xt, sums_next, hn_next, mo)
                    if h == 1 and next_xt is not None:
                        if mo == 0:
                            ps_g = gn_sum_mms(sums_next)
                            gn_finish(ps_g)
                        elif mo == 2:
                            ab_next[0] = gn_ab()
                        elif mo >= 4:
                            gn_hn_apply(next_xt, ab_next[0], hn_next, mo - 4)
                tail(h, ps_ua, ps_ub, zacc, ech_last)
            return U, Zb, hn_next

        def proj_mms(U, Zb, fuse=None):
            out_sb = opool.tile([128, CC, N], F32, tag="out")
            for h in range(NH):
                sl = bass.ts(h, 512)
                for co in range(CC):
                    ps_o = ps_pool.tile([128, 512], F32, tag="ps")
                    for ci in range(CC):
                        mm(ps_o[:], wo_sb[:, ci, bass.ts(co, 128)],
                           U[:, ci, sl], ci == 0, ci == CC - 1)
                    nc.vector.tensor_mul(out=out_sb[:, co, sl],
                                         in0=ps_o[:], in1=Zb[:, sl])
                    if fuse is not None:
                        nc.vector.tensor_add(out=out_sb[:, co, sl],
                                             in0=out_sb[:, co, sl],
                                             in1=fuse[:, co, sl])
                        if use_beff:
                            nc.vector.tensor_scalar_add(
                                out=out_sb[:, co, sl],
                                in0=out_sb[:, co, sl],
                                scalar1=beff_sb[:, co, :])
                        # per-chunk store: the tail drain only waits for the
                        # last 256KB instead of a 1MiB store
                        nc.gpsimd.dma_start(
                            out=outd.ap()[fuse_b[0]].rearrange(
                                "(cc p) n -> p cc n",
                                p=128)[:, co:co + 1, sl],
                            in_=out_sb[:, co:co + 1, sl])
            return out_sb

        def epilogue(out_sb, xt, b):
            for h in range(NH):
                sl = bass.ts(h, 512)
                for co in range(CC):
                    nc.vector.tensor_add(out=out_sb[:, co, sl],
                                         in0=out_sb[:, co, sl],
                                         in1=xt[:, co, sl])
                    if use_beff:
                        nc.vector.tensor_scalar_add(out=out_sb[:, co, sl],
                                                    in0=out_sb[:, co, sl],
                                                    scalar1=beff_sb[:, co, :])
                nc.gpsimd.dma_start(
                    out=outd.ap()[b].rearrange("(cc p) n -> p cc n",
                                               p=128)[:, :, sl],
                    in_=out_sb[:, :, sl])

        # ---- software-pipelined batch loop ----
        # GN of batch b+1 (stats, group matmuls, Newton rsqrt, broadcast,
        # affine apply) is emitted INSIDE attention(b), where PE/ACT/DVE
        # all have slack; batch boundaries carry only proj -> qkv.
        pending = None
        xt_cur = xt0
        hn_cur = None
        fuse_b = [None]
        for b in range(nbatch):
            if b == 0:
                ps_g = gn_sum_mms(sums0, dve_chunks=b0_dve_chunks)
                gn_finish(ps_g)
                hn_cur = gn_apply(xt_cur, hn0)
            out_prev = None
            if pending is not None:
                out_prev = proj_mms(pending[0], pending[1])
            qt, kt, vT = qkv(hn_cur)
            if pending is not None:
                epilogue(out_prev, pending[2], pending[3])
            xt_next = load_x(b + 1) if b + 1 < nbatch else None
            U, Zb, hn_next = attention(qt, kt, vT, next_xt=xt_next)
            pending = (U, Zb, xt_cur, b)
            xt_cur = xt_next
            hn_cur = hn_next
        fuse_b[0] = pending[3]
        proj_mms(pending[0], pending[1], fuse=pending[2])

    nc.compile()
    return nc


def make_host_inputs(x, gn_scale, gn_bias, wq, bq, wk, bk, wv, bv, wo, bo,
                     n_cores=8):
    """Shard + precompute host-side arrays. Returns (in_maps, nbatch)."""
    B = x.shape[0]
    nbatch = B // n_cores
    xr = np.ascontiguousarray(np.asarray(x, np.float32).reshape(B, C, N))
    beff = (np.asarray(wo, np.float32) @ np.asarray(bv, np.float32)
            + np.asarray(bo, np.float32))
    vpack = np.zeros((C, VP), np.float32)
    vpack[:, 0] = np.asarray(gn_scale, np.float32)
    vpack[:, 1] = np.asarray(gn_bias, np.float32)
    vpack[:, 2] = np.asarray(bq, np.float32)
    vpack[:, 3] = np.asarray(bk, np.float32)
    vpack[:, 4] = beff
    cidx = np.arange(C)
    vpack[cidx, 5 + cidx // GW] = 1.0 / (GW * N)
    vpack[cidx, 21 + cidx // GW] = 1.0 / GW
    indT = np.zeros((33, C), np.float32)
    indT[cidx // GW, cidx] = np.asarray(gn_scale, np.float32)
    indT[32, :] = np.asarray(gn_bias, np.float32)
    common = {
        "wqT": np.ascontiguousarray(np.asarray(wq, np.float32).T),
        "wkT": np.ascontiguousarray(np.asarray(wk, np.float32).T),
        "wvT": np.ascontiguousarray(np.asarray(wv, np.float32).T),
        "woT": np.ascontiguousarray(np.asarray(wo, np.float32).T),
        "vpack": vpack,
        "indT": indT,
        "ones": np.ones((128, 1), np.float32),
    }
    in_maps = []
    for i in range(n_cores):
        m = dict(common)
        m["xs"] = np.ascontiguousarray(xr[i * nbatch:(i + 1) * nbatch])
        in_maps.append(m)
    return in_maps, nbatch


_NC_CACHE = {}


def _get_nc(nbatch, use_beff):
    key = (nbatch, use_beff)
    if key not in _NC_CACHE:
        _NC_CACHE[key] = build_attention_nc(nbatch=nbatch, mm_dt="f32r",
                                            n_cores=8, use_beff=use_beff)
    return _NC_CACHE[key]


def kernel(x, gn_scale, gn_bias, wq, bq, wk, bk, wv, bv, wo, bo):
    """Full-input entry point: shards over 8 NeuronCores, returns full out."""
    from concourse.bass_utils import run_bass_kernel_spmd

    x = np.asarray(x, np.float32)
    B, Cin, H, W = x.shape
    assert (Cin, H * W) == (C, N), f"unexpected shape {x.shape}"
    n_cores = 8
    assert B % n_cores == 0
    in_maps, nbatch = make_host_inputs(
        x.reshape(B, C, N), gn_scale, gn_bias, wq, bq, wk, bk, wv, bv, wo, bo,
        n_cores=n_cores)
    use_beff = bool(np.any(in_maps[0]["vpack"][:, 4]))
    nc = _get_nc(nbatch, use_beff)
    res = run_bass_kernel_spmd(nc, in_maps, core_ids=list(range(n_cores)))
    out = np.concatenate([res.results[i]["out"] for i in range(n_cores)],
                         axis=0)
    return out.reshape(B, Cin, H, W).astype(np.float32)


# revision 5
# speedup vs baseline: 1.3266x; 1.0001x over previous
"""Self-contained Trainium2 Bass kernel for nn_AttentionBlock_80315888435976.

AttentionBlock: GroupNorm(16 groups) -> 1x1-conv q/k/v -> softmax attention
over the 32x32 spatial grid -> 1x1-conv out-projection -> residual.
Input x: [32, 512, 32, 32] fp32; weights [512, 512]; all biases [512].

Distribution: data-parallel over the batch dim across 8 NeuronCores
(4 batch elements per core); weights broadcast; no collectives.

Algebraic folds (host-side): scores = hn.T (wk.T wq) hn (q/k projections
collapse into one; the n-dependent bias terms cancel inside softmax), and
the out-projection commutes with the attention-weighted sum so the value
matrix is (wo @ wv) and the U accumulation directly yields the projected
output. v/out biases fold to a single per-channel constant wo@bv+bo.
"""
import sys
sys.path.insert(0, "/opt/trn_rl_repo")

import contextlib
import numpy as np

import concourse.bass as bass
import concourse.bass_isa as bass_isa
import concourse.bacc as bacc
import concourse.tile as tile
from concourse import mybir

F32 = mybir.dt.float32
F32R = mybir.dt.float32r
U32 = mybir.dt.uint32
AF = mybir.ActivationFunctionType
OP = mybir.AluOpType

C = 512
N = 1024
G = 16
GW = C // G      # 32 channels per group
CC = C // 128    # 4 channel chunks
NM = N // 128    # 8 m chunks
NH = N // 512    # 2 free halves
EPS = 1e-6
SCALE = 1.0 / np.sqrt(C)
# vecpack columns: 0 gnsc, 1 gnb, 2 bq, 3 bk, 4 beff,
#                  5:21 indm_sums (1/(GW*N)), 21:37 indm_mv (1/GW)
VP = 37
GE = 33        # gse rows: 0..15 = groups, 32 = bias row (base-partition
               # alignment: compute-engine APs must start at multiples of 32)


def build_attention_nc(nbatch=4, mm_dt="f32r", n_cores=8, use_beff=False,
                       use_qkb=False):
    nc = bacc.Bacc("TRN2", target_bir_lowering=False, debug=False,
                   num_devices=n_cores)
    rdt = F32R if mm_dt == "f32r" else F32

    xs = nc.dram_tensor("xs", [nbatch, C, N], F32, kind="ExternalInput")
    wqk = nc.dram_tensor("wqkT", [C, C], rdt, kind="ExternalInput")
    wv = nc.dram_tensor("wvT", [C, C], rdt, kind="ExternalInput")
    rvec = nc.dram_tensor("rvec", [C, 1], rdt, kind="ExternalInput")
    vpack = nc.dram_tensor("vpack", [C, VP], F32, kind="ExternalInput")
    indT = nc.dram_tensor("indT", [GE, C], F32, kind="ExternalInput")
    onesd = nc.dram_tensor("ones", [128, 1], rdt, kind="ExternalInput")
    outd = nc.dram_tensor("out", [nbatch, C, N], F32, kind="ExternalOutput")

    def r(dram2d):  # [C, X] dram -> [128, CC, X] view
        return dram2d.ap().rearrange("(cc p) x -> p cc x", p=128)

    def mm(ps, lhsT, rhs, start, stop):
        nc.tensor.matmul(ps, lhsT, rhs, start=start, stop=stop)

    with tile.TileContext(nc) as tc, contextlib.ExitStack() as ctx:
        wpool = ctx.enter_context(tc.tile_pool(name="w", bufs=1))
        vecs = ctx.enter_context(tc.tile_pool(name="vecs", bufs=1))
        xpool = ctx.enter_context(tc.tile_pool(name="x", bufs=3))
        hpool = ctx.enter_context(tc.tile_pool(name="hn", bufs=2))
        qkpool = ctx.enter_context(tc.tile_pool(name="qk", bufs=1))
        vpool = ctx.enter_context(tc.tile_pool(name="v", bufs=1))
        epool = ctx.enter_context(tc.tile_pool(name="e", bufs=3))
        upool = ctx.enter_context(tc.tile_pool(name="u", bufs=1))
        opool = ctx.enter_context(tc.tile_pool(name="o", bufs=1))
        rpool = ctx.enter_context(tc.tile_pool(name="r", bufs=2))
        stats = ctx.enter_context(tc.tile_pool(name="st", bufs=2))
        ps_pool = ctx.enter_context(tc.tile_pool(name="ps", bufs=4, space="PSUM"))
        acc_pool = ctx.enter_context(tc.tile_pool(name="acc", bufs=1, space="PSUM"))

        # ---- constants (3 DMAs; HWDGE issue pipe costs ~0.65us per DMA) ----
        vp_sb = vecs.tile([128, CC, VP], F32, tag="vp")
        indT_sb = vecs.tile([GE, CC, 128], F32, tag="indT")
        ones_sb = vecs.tile([128, 1], rdt, tag="ones")
        gse = vecs.tile([GE, 2], F32, tag="gse")
        eps_sb = vecs.tile([G, 1], F32, tag="eps")
        magic_sb = vecs.tile([G, 1], U32, tag="magic")
        c15_sb = vecs.tile([G, 1], F32, tag="c15")
        nc.vector.memset(eps_sb[:], EPS)
        nc.vector.memset(magic_sb[:], 0x5f3759df)
        nc.vector.memset(c15_sb[:], 1.5)
        nc.vector.memset(gse[32:GE, 0:1], 0.0)
        nc.vector.memset(gse[32:GE, 1:2], 1.0)
        gnsc_sb = vp_sb[:, :, 0:1]
        gnb_sb = vp_sb[:, :, 1:2]
        bq_sb = vp_sb[:, :, 2:3]
        bk_sb = vp_sb[:, :, 3:4]
        beff_sb = vp_sb[:, :, 4:5]

        def stat_op(xt, sums, scr, k):
            """k-th of 8 ACT ops accumulating per-channel sum / sum-sq.
            scr is a scratch dummy output (only accum_out matters)."""
            cc, which = divmod(k, 2)
            nc.scalar.activation(out=scr[:, cc, :],
                                 in_=xt[:, cc, :],
                                 func=(AF.Copy if which == 0 else AF.Square),
                                 accum_out=sums[:, cc, which:which + 1])

        def gn_stat_tiles():
            # (scr is not allocated here: the next batch's hn tile doubles as
            # the dummy activation output until its real write in h1)
            return stats.tile([128, CC, 2], F32, tag="sums", name="sums")

        def gn_sum_mms(sums, dve_chunks=()):
            ps_g = ps_pool.tile([G, 2], F32, tag="ps")
            for cc in range(CC):
                col = slice(21, 37) if cc in dve_chunks else slice(5, 21)
                nc.tensor.matmul(ps_g[:], vp_sb[:, cc, col], sums[:, cc, :],
                                 start=(cc == 0), stop=(cc == CC - 1))
            return ps_g

        # ---- batch-0 x load: per chunk, stats split ACT/DVE ----
        xt0 = xpool.tile([128, CC, N], F32, tag="x", name="xt0")
        hn0 = hpool.tile([128, CC, N], F32R if mm_dt == "f32r" else F32,
                         tag="hn", name="hn0")
        sums0 = gn_stat_tiles()
        st6_0 = stats.tile([128, CC, 2, 6], F32, tag="st6")
        mv0 = stats.tile([128, CC, 2], F32, tag="mv")
        b0_dve_chunks = (1, 3)
        for cc in range(CC):
            nc.sync.dma_start(out=xt0[:, cc, :],
                              in_=xs.ap()[0][bass.ts(cc, 128), :])
            if cc in b0_dve_chunks:
                # DVE path -> sums0[:, cc] = [mu_c, mu_c^2 + var_c]
                for h in range(2):
                    nc.vector.bn_stats(out=st6_0[:, cc, h, :],
                                       in_=xt0[:, cc, bass.ts(h, 512)])
                nc.vector.bn_aggr(out=mv0[:, cc, :], in_=st6_0[:, cc, :, :])
                nc.vector.tensor_mul(out=sums0[:, cc, 1:2],
                                     in0=mv0[:, cc, 0:1], in1=mv0[:, cc, 0:1])
                nc.vector.tensor_add(out=sums0[:, cc, 1:2],
                                     in0=sums0[:, cc, 1:2], in1=mv0[:, cc, 1:2])
                nc.vector.tensor_copy(out=sums0[:, cc, 0:1],
                                      in_=mv0[:, cc, 0:1])
            else:
                stat_op(xt0, sums0, hn0, 2 * cc)
                stat_op(xt0, sums0, hn0, 2 * cc + 1)

        nc.sync.dma_start(out=vp_sb[:], in_=r(vpack))
        nc.sync.dma_start(
            out=indT_sb[:], in_=indT.ap().rearrange("g (cc p) -> g cc p", p=128))
        nc.sync.dma_start(out=ones_sb[:], in_=onesd.ap())

        wqk_sb = wpool.tile([128, CC, C], rdt, tag="wqk")
        wv_sb = wpool.tile([128, CC, C], rdt, tag="wv")
        nc.sync.dma_start(out=wqk_sb[:], in_=r(wqk))
        nc.sync.dma_start(out=wv_sb[:], in_=r(wv))
        rv_sb = None
        if use_qkb:
            rv_sb = vecs.tile([128, CC, 1], rdt, tag="rv")
            nc.sync.dma_start(out=rv_sb[:], in_=r(rvec))

        def load_x(b):
            xt = xpool.tile([128, CC, N], F32, tag="x")
            nc.sync.dma_start(
                out=xt[:], in_=xs.ap()[b].rearrange("(cc p) n -> p cc n", p=128))
            return xt

        def gn_finish(ps_g):
            """[mu_g, m2_g] -> gse rows 0..15 = [rstd_g, -mu_g*rstd_g]."""
            gsb = stats.tile([G, 2], F32, tag="gsb")
            varg = stats.tile([G, 1], F32, tag="varg")
            nc.vector.tensor_copy(out=gsb[:], in_=ps_g[:])
            nc.vector.tensor_mul(out=varg[:], in0=gsb[:, 0:1], in1=gsb[:, 0:1])
            nc.vector.tensor_tensor(out=varg[:], in0=gsb[:, 1:2], in1=varg[:],
                                    op=OP.subtract)
            nc.vector.tensor_scalar_add(out=varg[:], in0=varg[:], scalar1=EPS)
            y = stats.tile([G, 1], F32, tag="nwt_y")
            vh = stats.tile([G, 1], F32, tag="nwt_vh")
            t = stats.tile([G, 1], F32, tag="nwt_t")
            nc.vector.tensor_scalar(out=t[:].bitcast(U32),
                                    in0=varg[:].bitcast(U32),
                                    scalar1=1, scalar2=None,
                                    op0=OP.logical_shift_right)
            nc.vector.tensor_tensor(out=y[:].bitcast(U32), in0=magic_sb[:],
                                    in1=t[:].bitcast(U32), op=OP.subtract)
            nc.vector.tensor_scalar_mul(out=vh[:], in0=varg[:], scalar1=0.5)
            for it in range(2):
                nc.vector.tensor_mul(out=t[:], in0=y[:], in1=y[:])
                nc.vector.tensor_mul(out=t[:], in0=vh[:], in1=t[:])
                nc.vector.tensor_tensor(out=t[:], in0=c15_sb[:], in1=t[:],
                                        op=OP.subtract)
                dst = gse[0:G, 0:1] if it == 1 else y[:]
                nc.vector.tensor_mul(out=dst, in0=y[:], in1=t[:])
            nc.vector.tensor_mul(out=t[:], in0=gsb[:, 0:1], in1=gse[0:G, 0:1])
            nc.vector.tensor_scalar_mul(out=gse[0:G, 1:2], in0=t[:],
                                        scalar1=-1.0)

        def gn_ab(dve=False):
            ab_sb = stats.tile([128, CC, 2], F32, tag="ab_sb")
            for cc in range(CC):
                ps_cb = ps_pool.tile([128, 2], F32, tag="ps")
                nc.tensor.matmul(ps_cb[:], indT_sb[:, cc, :], gse[:],
                                 start=True, stop=True)
                if dve:
                    nc.vector.tensor_copy(out=ab_sb[:, cc, :], in_=ps_cb[:])
                else:
                    nc.scalar.activation(out=ab_sb[:, cc, :], in_=ps_cb[:],
                                         func=AF.Copy)
            return ab_sb

        def gn_hn_apply(xt, ab_sb, hn, cc, dve_extra=False):
            if cc % 2 or (dve_extra and cc == 2):
                nc.vector.tensor_scalar(out=hn[:, cc, :], in0=xt[:, cc, :],
                                        scalar1=ab_sb[:, cc, 0:1],
                                        scalar2=ab_sb[:, cc, 1:2],
                                        op0=OP.mult, op1=OP.add)
            else:
                nc.scalar.activation(out=hn[:, cc, :], in_=xt[:, cc, :],
                                     func=AF.Identity,
                                     scale=ab_sb[:, cc, 0:1],
                                     bias=ab_sb[:, cc, 1:2])

        def gn_apply(xt, hn):
            ab_sb = gn_ab(dve=True)
            for cc in range(CC):
                gn_hn_apply(xt, ab_sb, hn, cc)
            return hn

        def qkv(hn):
            """kq = (wk.T wq) @ hn  (q and k fold into one projection: the
            softmax over m is invariant to per-n additive constants).
            vT = hn.T @ (wo@wv).T.  With nonzero bq, the m-dependent score
            bias r[m] = (wk.T bq).hn[:,m] is accumulated for use as a
            per-partition exp bias."""
            kqt = qkpool.tile([128, CC, N], rdt, tag="kq")
            for co in range(CC):
                for h in range(NH):
                    ps_t = ps_pool.tile([128, 512], F32, tag="ps")
                    for ci in range(CC):
                        mm(ps_t[:], wqk_sb[:, ci, bass.ts(co, 128)],
                           hn[:, ci, bass.ts(h, 512)], ci == 0, ci == CC - 1)
                    nc.vector.tensor_copy(out=kqt[:, co, bass.ts(h, 512)],
                                          in_=ps_t[:])
            vT = vpool.tile([128, NM, C], rdt, tag="vT")
            rt = None
            if use_qkb:
                rt = stats.tile([128, NM, 1], F32, tag="rt")
            for mo in range(NM):
                ps_t = ps_pool.tile([128, 512], F32, tag="ps")
                for ci in range(CC):
                    mm(ps_t[:], hn[:, ci, bass.ts(mo, 128)], wv_sb[:, ci, :],
                       ci == 0, ci == CC - 1)
                nc.vector.tensor_copy(out=vT[:, mo, :], in_=ps_t[:])
                if use_qkb:
                    ps_r = ps_pool.tile([128, 1], F32, tag="ps")
                    for ci in range(CC):
                        nc.tensor.matmul(ps_r[:],
                                         hn[:, ci, bass.ts(mo, 128)],
                                         rv_sb[:, ci, :],
                                         start=(ci == 0), stop=(ci == CC - 1))
                    nc.vector.tensor_copy(out=rt[:, mo, :], in_=ps_r[:])
            return kqt, vT, rt

        def attention(hn, kqt, vT, rt, next_xt=None):
            """vT is hn.T @ (wo@wv).T: the U accumulation directly yields the
            unnormalized out-projection; normalize/residual run deferred in
            the next batch's qkv window (see epilogue)."""
            Zb = rpool.tile([128, N], F32, tag="Zb")
            U = upool.tile([128, CC, N], rdt, tag="U")
            sums_next = hn_next = None
            ab_next = [None]
            if next_xt is not None:
                sums_next = gn_stat_tiles()
                hn_next = hpool.tile([128, CC, N], rdt, tag="hn", name="hn")

            def emit_scores(h, mo):
                ps_s = ps_pool.tile([128, 512], F32, tag="ps", name="ps_s")
                for ci in range(CC):
                    mm(ps_s[:], hn[:, ci, bass.ts(mo, 128)],
                       kqt[:, ci, bass.ts(h, 512)], ci == 0, ci == CC - 1)
                ech = epool.tile([128, 512], rdt, tag="e", name="ech")
                if use_qkb:
                    nc.scalar.activation(out=ech[:], in_=ps_s[:], func=AF.Exp,
                                         scale=SCALE, bias=rt[:, mo, :])
                else:
                    nc.scalar.activation(out=ech[:], in_=ps_s[:], func=AF.Exp,
                                         scale=SCALE)
                return ech

            sched = [(h, mo) for h in range(NH) for mo in range(NM)]
            emitted = {}
            ptr = [0]

            def ensure(upto):
                while ptr[0] < len(sched) and ptr[0] <= upto:
                    hh, mm_ = sched[ptr[0]]
                    emitted[(hh, mm_)] = emit_scores(hh, mm_)
                    ptr[0] += 1

            def tail(h, ps_ua, ps_ub, zacc, ech_last):
                sl = bass.ts(h, 512)
                for co in range(CC):
                    pu = ps_ua if co < 2 else ps_ub
                    nc.scalar.activation(out=U[:, co, sl],
                                         in_=pu[:, co % 2, :], func=AF.Copy)
                zp = rpool.tile([128, 512], F32, tag="zp", name="zp")
                nc.gpsimd.partition_all_reduce(
                    zp[:], ech_last[:].bitcast(F32), channels=128,
                    reduce_op=bass_isa.ReduceOp.add)
                nc.vector.tensor_add(out=zacc[:], in0=zacc[:], in1=zp[:])
                nc.vector.reciprocal(out=Zb[:, sl], in_=zacc[:])

            ensure(1)
            for h in range(NH):
                zacc = rpool.tile([128, 512], F32, tag="zacc", name="zacc")
                ps_ua = ps_ub = None
                ech_last = None
                for mo in range(NM):
                    i = h * NM + mo
                    ensure(i + 1)
                    ech = emitted.pop((h, mo))
                    if mo == 0:
                        ps_ua = acc_pool.tile([128, 2, 512], F32, tag="acca",
                                              name="ps_ua")
                        ps_ub = acc_pool.tile([128, 2, 512], F32, tag="accb",
                                              name="ps_ub")
                        nc.gpsimd.partition_all_reduce(
                            zacc[:], ech[:].bitcast(F32), channels=128,
                            reduce_op=bass_isa.ReduceOp.add)
                    elif mo < NM - 1:
                        zp = rpool.tile([128, 512], F32, tag="zp", name="zp")
                        nc.gpsimd.partition_all_reduce(
                            zp[:], ech[:].bitcast(F32), channels=128,
                            reduce_op=bass_isa.ReduceOp.add)
                        nc.vector.tensor_add(out=zacc[:], in0=zacc[:],
                                             in1=zp[:])
                    else:
                        ech_last = ech  # z-accumulate deferred past U copies
                    for co in range(CC):
                        pu = ps_ua if co < 2 else ps_ub
                        mm(pu[:, co % 2, :], vT[:, mo, bass.ts(co, 128)],
                           ech[:], mo == 0, mo == NM - 1)
                    if h == 0 and next_xt is not None:
                        stat_op(next_xt, sums_next, hn_next, mo)
                    if h == 1 and next_xt is not None:
                        if mo == 0:
                            ps_g = gn_sum_mms(sums_next)
                            gn_finish(ps_g)
                        elif mo == 2:
                            ab_next[0] = gn_ab()
                        elif mo >= 4:
                            gn_hn_apply(next_xt, ab_next[0], hn_next, mo - 4)
                tail(h, ps_ua, ps_ub, zacc, ech_last)
            return U, Zb, hn_next

        def epilogue(U, Zb, xt, b):
            """Deferred normalize + residual + store (DVE work, emitted in
            the next batch's qkv window where DVE is otherwise light)."""
            out_sb = opool.tile([128, CC, N], F32, tag="out")
            for h in range(NH):
                sl = bass.ts(h, 512)
                for co in range(CC):
                    nc.vector.tensor_mul(out=out_sb[:, co, sl],
                                         in0=U[:, co, sl], in1=Zb[:, sl])
                    nc.vector.tensor_add(out=out_sb[:, co, sl],
                                         in0=out_sb[:, co, sl],
                                         in1=xt[:, co, sl])
                    if use_beff:
                        nc.vector.tensor_scalar_add(out=out_sb[:, co, sl],
                                                    in0=out_sb[:, co, sl],
                                                    scalar1=beff_sb[:, co, :])
                    eng = nc.sync if co % 2 == 0 else nc.gpsimd
                    eng.dma_start(
                        out=outd.ap()[b].rearrange(
                            "(cc p) n -> p cc n", p=128)[:, co:co + 1, sl],
                        in_=out_sb[:, co:co + 1, sl])

        # ---- software-pipelined batch loop ----
        pending = None
        xt_cur = xt0
        hn_cur = None
        for b in range(nbatch):
            if b == 0:
                ps_g = gn_sum_mms(sums0, dve_chunks=b0_dve_chunks)
                gn_finish(ps_g)
                hn_cur = gn_apply(xt_cur, hn0)
            kqt, vT, rt = qkv(hn_cur)
            if pending is not None:
                epilogue(*pending)
            xt_next = load_x(b + 1) if b + 1 < nbatch else None
            U, Zb, hn_next = attention(hn_cur, kqt, vT, rt, next_xt=xt_next)
            pending = (U, Zb, xt_cur, b)
            xt_cur = xt_next
            hn_cur = hn_next
        epilogue(*pending)

    nc.compile()
    return nc


def make_host_inputs(x, gn_scale, gn_bias, wq, bq, wk, bk, wv, bv, wo, bo,
                     n_cores=8):
    """Shard + precompute host-side arrays. Returns (in_maps, nbatch)."""
    B = x.shape[0]
    nbatch = B // n_cores
    xr = np.ascontiguousarray(np.asarray(x, np.float32).reshape(B, C, N))
    beff = (np.asarray(wo, np.float32) @ np.asarray(bv, np.float32)
            + np.asarray(bo, np.float32))
    vpack = np.zeros((C, VP), np.float32)
    vpack[:, 0] = np.asarray(gn_scale, np.float32)
    vpack[:, 1] = np.asarray(gn_bias, np.float32)
    vpack[:, 2] = np.asarray(bq, np.float32)
    vpack[:, 3] = np.asarray(bk, np.float32)
    vpack[:, 4] = beff
    cidx = np.arange(C)
    vpack[cidx, 5 + cidx // GW] = 1.0 / (GW * N)
    vpack[cidx, 21 + cidx // GW] = 1.0 / GW
    indT = np.zeros((33, C), np.float32)
    indT[cidx // GW, cidx] = np.asarray(gn_scale, np.float32)
    indT[32, :] = np.asarray(gn_bias, np.float32)
    wqf = np.asarray(wq, np.float32)
    wkf = np.asarray(wk, np.float32)
    common = {
        # q and k projections fold into one: scores = hn.T (wk.T wq) hn
        "wqkT": np.ascontiguousarray(wqf.T @ wkf),
        # m-dependent score bias from bq (zero-bias case: unused)
        "rvec": ((wkf.T @ np.asarray(bq, np.float32)).reshape(C, 1)
                 / np.sqrt(np.float32(C))).astype(np.float32),
        # wo is folded into the value projection: the attention-weighted sum
        # commutes with the (linear) out-projection
        "wvT": np.ascontiguousarray(
            (np.asarray(wo, np.float32) @ np.asarray(wv, np.float32)).T),
        "vpack": vpack,
        "indT": indT,
        "ones": np.ones((128, 1), np.float32),
    }
    in_maps = []
    for i in range(n_cores):
        m = dict(common)
        m["xs"] = np.ascontiguousarray(xr[i * nbatch:(i + 1) * nbatch])
        in_maps.append(m)
    return in_maps, nbatch


_NC_CACHE = {}


def _get_nc(nbatch, use_beff, use_qkb):
    key = (nbatch, use_beff, use_qkb)
    if key not in _NC_CACHE:
        _NC_CACHE[key] = build_attention_nc(nbatch=nbatch, mm_dt="f32r",
                                            n_cores=8, use_beff=use_beff,
                                            use_qkb=use_qkb)
    return _NC_CACHE[key]


def kernel(x, gn_scale, gn_bias, wq, bq, wk, bk, wv, bv, wo, bo):
    """Full-input entry point: shards over 8 NeuronCores, returns full out."""
    from concourse.bass_utils import run_bass_kernel_spmd

    x = np.asarray(x, np.float32)
    B, Cin, H, W = x.shape
    assert (Cin, H * W) == (C, N), f"unexpected shape {x.shape}"
    n_cores = 8
    assert B % n_cores == 0
    in_maps, nbatch = make_host_inputs(
        x.reshape(B, C, N), gn_scale, gn_bias, wq, bq, wk, bk, wv, bv, wo, bo,
        n_cores=n_cores)
    use_beff = bool(np.any(in_maps[0]["vpack"][:, 4]))
    use_qkb = bool(np.any(np.asarray(bq, np.float32)))
    nc = _get_nc(nbatch, use_beff, use_qkb)
    res = run_bass_kernel_spmd(nc, in_maps, core_ids=list(range(n_cores)))
    out = np.concatenate([res.results[i]["out"] for i in range(n_cores)],
                         axis=0)
    return out.reshape(B, Cin, H, W).astype(np.float32)


# revision 6
# speedup vs baseline: 1.3282x; 1.0012x over previous
"""Self-contained Trainium2 Bass kernel for nn_AttentionBlock_80315888435976.

AttentionBlock: GroupNorm(16 groups) -> 1x1-conv q/k/v -> softmax attention
over the 32x32 spatial grid -> 1x1-conv out-projection -> residual.
Input x: [32, 512, 32, 32] fp32; weights [512, 512]; all biases [512].

Distribution: data-parallel over the batch dim across 8 NeuronCores
(4 batch elements per core); weights broadcast; no collectives.

Algebraic folds (host-side): scores = hn.T (wk.T wq) hn (q/k projections
collapse into one; the n-dependent bias terms cancel inside softmax), and
the out-projection commutes with the attention-weighted sum so the value
matrix is (wo @ wv) and the U accumulation directly yields the projected
output. v/out biases fold to a single per-channel constant wo@bv+bo.
"""
import sys
sys.path.insert(0, "/opt/trn_rl_repo")

import contextlib
import numpy as np

import concourse.bass as bass
import concourse.bass_isa as bass_isa
import concourse.bacc as bacc
import concourse.tile as tile
from concourse import mybir

F32 = mybir.dt.float32
F32R = mybir.dt.float32r
U32 = mybir.dt.uint32
AF = mybir.ActivationFunctionType
OP = mybir.AluOpType

C = 512
N = 1024
G = 16
GW = C // G      # 32 channels per group
CC = C // 128    # 4 channel chunks
NM = N // 128    # 8 m chunks
NH = N // 512    # 2 free halves
EPS = 1e-6
SCALE = 1.0 / np.sqrt(C)
# vecpack columns: 0 gnsc, 1 gnb, 2 bq, 3 bk, 4 beff,
#                  5:21 indm_sums (1/(GW*N)), 21:37 indm_mv (1/GW)
VP = 37
GE = 33        # gse rows: 0..15 = groups, 32 = bias row (base-partition
               # alignment: compute-engine APs must start at multiples of 32)


def build_attention_nc(nbatch=4, mm_dt="f32r", n_cores=8, use_beff=False,
                       use_qkb=False):
    nc = bacc.Bacc("TRN2", target_bir_lowering=False, debug=False,
                   num_devices=n_cores)
    rdt = F32R if mm_dt == "f32r" else F32

    xs = nc.dram_tensor("xs", [nbatch, C, N], F32, kind="ExternalInput")
    wqk = nc.dram_tensor("wqkT", [C, C], rdt, kind="ExternalInput")
    wv = nc.dram_tensor("wvT", [C, C], rdt, kind="ExternalInput")
    rvec = nc.dram_tensor("rvec", [C, 1], rdt, kind="ExternalInput")
    vpack = nc.dram_tensor("vpack", [C, VP], F32, kind="ExternalInput")
    indT = nc.dram_tensor("indT", [GE, C], F32, kind="ExternalInput")
    onesd = nc.dram_tensor("ones", [128, 1], rdt, kind="ExternalInput")
    outd = nc.dram_tensor("out", [nbatch, C, N], F32, kind="ExternalOutput")

    def r(dram2d):  # [C, X] dram -> [128, CC, X] view
        return dram2d.ap().rearrange("(cc p) x -> p cc x", p=128)

    def mm(ps, lhsT, rhs, start, stop):
        nc.tensor.matmul(ps, lhsT, rhs, start=start, stop=stop)

    with tile.TileContext(nc) as tc, contextlib.ExitStack() as ctx:
        wpool = ctx.enter_context(tc.tile_pool(name="w", bufs=1))
        vecs = ctx.enter_context(tc.tile_pool(name="vecs", bufs=1))
        xpool = ctx.enter_context(tc.tile_pool(name="x", bufs=3))
        hpool = ctx.enter_context(tc.tile_pool(name="hn", bufs=2))
        qkpool = ctx.enter_context(tc.tile_pool(name="qk", bufs=1))
        vpool = ctx.enter_context(tc.tile_pool(name="v", bufs=1))
        epool = ctx.enter_context(tc.tile_pool(name="e", bufs=3))
        upool = ctx.enter_context(tc.tile_pool(name="u", bufs=1))
        opool = ctx.enter_context(tc.tile_pool(name="o", bufs=1))
        rpool = ctx.enter_context(tc.tile_pool(name="r", bufs=2))
        stats = ctx.enter_context(tc.tile_pool(name="st", bufs=2))
        ps_pool = ctx.enter_context(tc.tile_pool(name="ps", bufs=4, space="PSUM"))
        acc_pool = ctx.enter_context(tc.tile_pool(name="acc", bufs=1, space="PSUM"))

        # ---- constants (3 DMAs; HWDGE issue pipe costs ~0.65us per DMA) ----
        vp_sb = vecs.tile([128, CC, VP], F32, tag="vp")
        indT_sb = vecs.tile([GE, CC, 128], F32, tag="indT")
        ones_sb = vecs.tile([128, 1], rdt, tag="ones")
        gse = vecs.tile([GE, 2], F32, tag="gse")
        eps_sb = vecs.tile([G, 1], F32, tag="eps")
        magic_sb = vecs.tile([G, 1], U32, tag="magic")
        c15_sb = vecs.tile([G, 1], F32, tag="c15")
        nc.vector.memset(eps_sb[:], EPS)
        nc.vector.memset(magic_sb[:], 0x5f3759df)
        nc.vector.memset(c15_sb[:], 1.5)
        nc.vector.memset(gse[32:GE, 0:1], 0.0)
        nc.vector.memset(gse[32:GE, 1:2], 1.0)
        gnsc_sb = vp_sb[:, :, 0:1]
        gnb_sb = vp_sb[:, :, 1:2]
        bq_sb = vp_sb[:, :, 2:3]
        bk_sb = vp_sb[:, :, 3:4]
        beff_sb = vp_sb[:, :, 4:5]

        def stat_op(xt, sums, scr, k):
            """k-th of 8 ACT ops accumulating per-channel sum / sum-sq.
            scr is a scratch dummy output (only accum_out matters)."""
            cc, which = divmod(k, 2)
            nc.scalar.activation(out=scr[:, cc, :],
                                 in_=xt[:, cc, :],
                                 func=(AF.Copy if which == 0 else AF.Square),
                                 accum_out=sums[:, cc, which:which + 1])

        def gn_stat_tiles():
            # (scr is not allocated here: the next batch's hn tile doubles as
            # the dummy activation output until its real write in h1)
            return stats.tile([128, CC, 2], F32, tag="sums", name="sums")

        def gn_sum_mms(sums, dve_chunks=()):
            ps_g = ps_pool.tile([G, 2], F32, tag="ps")
            for cc in range(CC):
                col = slice(21, 37) if cc in dve_chunks else slice(5, 21)
                nc.tensor.matmul(ps_g[:], vp_sb[:, cc, col], sums[:, cc, :],
                                 start=(cc == 0), stop=(cc == CC - 1))
            return ps_g

        # ---- batch-0 x load: per chunk, stats split ACT/DVE ----
        xt0 = xpool.tile([128, CC, N], F32, tag="x", name="xt0")
        hn0 = hpool.tile([128, CC, N], F32R if mm_dt == "f32r" else F32,
                         tag="hn", name="hn0")
        sums0 = gn_stat_tiles()
        st6_0 = stats.tile([128, CC, 2, 6], F32, tag="st6")
        mv0 = stats.tile([128, CC, 2], F32, tag="mv")
        b0_dve_chunks = (1, 3)
        for cc in range(CC):
            nc.sync.dma_start(out=xt0[:, cc, :],
                              in_=xs.ap()[0][bass.ts(cc, 128), :])
            if cc in b0_dve_chunks:
                # DVE path -> sums0[:, cc] = [mu_c, mu_c^2 + var_c]
                for h in range(2):
                    nc.vector.bn_stats(out=st6_0[:, cc, h, :],
                                       in_=xt0[:, cc, bass.ts(h, 512)])
                nc.vector.bn_aggr(out=mv0[:, cc, :], in_=st6_0[:, cc, :, :])
                nc.vector.tensor_mul(out=sums0[:, cc, 1:2],
                                     in0=mv0[:, cc, 0:1], in1=mv0[:, cc, 0:1])
                nc.vector.tensor_add(out=sums0[:, cc, 1:2],
                                     in0=sums0[:, cc, 1:2], in1=mv0[:, cc, 1:2])
                nc.vector.tensor_copy(out=sums0[:, cc, 0:1],
                                      in_=mv0[:, cc, 0:1])
            else:
                stat_op(xt0, sums0, hn0, 2 * cc)
                stat_op(xt0, sums0, hn0, 2 * cc + 1)

        nc.sync.dma_start(out=vp_sb[:], in_=r(vpack))
        nc.sync.dma_start(
            out=indT_sb[:], in_=indT.ap().rearrange("g (cc p) -> g cc p", p=128))
        nc.sync.dma_start(out=ones_sb[:], in_=onesd.ap())

        wqk_sb = wpool.tile([128, CC, C], rdt, tag="wqk")
        wv_sb = wpool.tile([128, CC, C], rdt, tag="wv")
        nc.sync.dma_start(out=wqk_sb[:], in_=r(wqk))
        nc.sync.dma_start(out=wv_sb[:], in_=r(wv))
        rv_sb = None
        if use_qkb:
            rv_sb = vecs.tile([128, CC, 1], rdt, tag="rv")
            nc.sync.dma_start(out=rv_sb[:], in_=r(rvec))

        def load_x(b):
            xt = xpool.tile([128, CC, N], F32, tag="x")
            nc.sync.dma_start(
                out=xt[:], in_=xs.ap()[b].rearrange("(cc p) n -> p cc n", p=128))
            return xt

        def gn_finish(ps_g):
            """[mu_g, m2_g] -> gse rows 0..15 = [rstd_g, -mu_g*rstd_g]."""
            gsb = stats.tile([G, 2], F32, tag="gsb")
            varg = stats.tile([G, 1], F32, tag="varg")
            nc.vector.tensor_copy(out=gsb[:], in_=ps_g[:])
            nc.vector.tensor_mul(out=varg[:], in0=gsb[:, 0:1], in1=gsb[:, 0:1])
            nc.vector.tensor_tensor(out=varg[:], in0=gsb[:, 1:2], in1=varg[:],
                                    op=OP.subtract)
            nc.vector.tensor_scalar_add(out=varg[:], in0=varg[:], scalar1=EPS)
            y = stats.tile([G, 1], F32, tag="nwt_y")
            vh = stats.tile([G, 1], F32, tag="nwt_vh")
            t = stats.tile([G, 1], F32, tag="nwt_t")
            nc.vector.tensor_scalar(out=t[:].bitcast(U32),
                                    in0=varg[:].bitcast(U32),
                                    scalar1=1, scalar2=None,
                                    op0=OP.logical_shift_right)
            nc.vector.tensor_tensor(out=y[:].bitcast(U32), in0=magic_sb[:],
                                    in1=t[:].bitcast(U32), op=OP.subtract)
            nc.vector.tensor_scalar_mul(out=vh[:], in0=varg[:], scalar1=0.5)
            for it in range(2):
                nc.vector.tensor_mul(out=t[:], in0=y[:], in1=y[:])
                nc.vector.tensor_mul(out=t[:], in0=vh[:], in1=t[:])
                nc.vector.tensor_tensor(out=t[:], in0=c15_sb[:], in1=t[:],
                                        op=OP.subtract)
                dst = gse[0:G, 0:1] if it == 1 else y[:]
                nc.vector.tensor_mul(out=dst, in0=y[:], in1=t[:])
            nc.vector.tensor_mul(out=t[:], in0=gsb[:, 0:1], in1=gse[0:G, 0:1])
            nc.vector.tensor_scalar_mul(out=gse[0:G, 1:2], in0=t[:],
                                        scalar1=-1.0)

        def gn_ab(dve=False):
            ab_sb = stats.tile([128, CC, 2], F32, tag="ab_sb")
            for cc in range(CC):
                ps_cb = ps_pool.tile([128, 2], F32, tag="ps")
                nc.tensor.matmul(ps_cb[:], indT_sb[:, cc, :], gse[:],
                                 start=True, stop=True)
                if dve:
                    nc.vector.tensor_copy(out=ab_sb[:, cc, :], in_=ps_cb[:])
                else:
                    nc.scalar.activation(out=ab_sb[:, cc, :], in_=ps_cb[:],
                                         func=AF.Copy)
            return ab_sb

        def gn_hn_apply(xt, ab_sb, hn, cc, dve_extra=False):
            if cc % 2 or (dve_extra and cc == 2):
                nc.vector.tensor_scalar(out=hn[:, cc, :], in0=xt[:, cc, :],
                                        scalar1=ab_sb[:, cc, 0:1],
                                        scalar2=ab_sb[:, cc, 1:2],
                                        op0=OP.mult, op1=OP.add)
            else:
                nc.scalar.activation(out=hn[:, cc, :], in_=xt[:, cc, :],
                                     func=AF.Identity,
                                     scale=ab_sb[:, cc, 0:1],
                                     bias=ab_sb[:, cc, 1:2])

        def gn_apply(xt, hn):
            ab_sb = gn_ab(dve=True)
            for cc in range(CC):
                gn_hn_apply(xt, ab_sb, hn, cc)
            return hn

        def qkv(hn):
            """kq = (wk.T wq) @ hn  (q and k fold into one projection: the
            softmax over m is invariant to per-n additive constants).
            vT = hn.T @ (wo@wv).T.  With nonzero bq, the m-dependent score
            bias r[m] = (wk.T bq).hn[:,m] is accumulated for use as a
            per-partition exp bias."""
            kqt = qkpool.tile([128, CC, N], rdt, tag="kq")
            for co in range(CC):
                for h in range(NH):
                    ps_t = ps_pool.tile([128, 512], F32, tag="ps")
                    for ci in range(CC):
                        mm(ps_t[:], wqk_sb[:, ci, bass.ts(co, 128)],
                           hn[:, ci, bass.ts(h, 512)], ci == 0, ci == CC - 1)
                    nc.vector.tensor_copy(out=kqt[:, co, bass.ts(h, 512)],
                                          in_=ps_t[:])
            vT = vpool.tile([128, NM, C], rdt, tag="vT")
            rt = None
            if use_qkb:
                rt = stats.tile([128, NM, 1], F32, tag="rt")
            for mo in range(NM):
                ps_t = ps_pool.tile([128, 512], F32, tag="ps")
                for ci in range(CC):
                    mm(ps_t[:], hn[:, ci, bass.ts(mo, 128)], wv_sb[:, ci, :],
                       ci == 0, ci == CC - 1)
                nc.vector.tensor_copy(out=vT[:, mo, :], in_=ps_t[:])
                if use_qkb:
                    ps_r = ps_pool.tile([128, 1], F32, tag="ps")
                    for ci in range(CC):
                        nc.tensor.matmul(ps_r[:],
                                         hn[:, ci, bass.ts(mo, 128)],
                                         rv_sb[:, ci, :],
                                         start=(ci == 0), stop=(ci == CC - 1))
                    nc.vector.tensor_copy(out=rt[:, mo, :], in_=ps_r[:])
            return kqt, vT, rt

        def attention(hn, kqt, vT, rt, next_xt=None, fuse_xt_b=None):
            """vT is hn.T @ (wo@wv).T: the U accumulation directly yields the
            unnormalized out-projection; normalize/residual run deferred in
            the next batch's qkv window (see epilogue)."""
            Zb = rpool.tile([128, N], F32, tag="Zb")
            U = upool.tile([128, CC, N], rdt, tag="U")
            sums_next = hn_next = None
            ab_next = [None]
            if next_xt is not None:
                sums_next = gn_stat_tiles()
                hn_next = hpool.tile([128, CC, N], rdt, tag="hn", name="hn")

            def emit_scores(h, mo):
                ps_s = ps_pool.tile([128, 512], F32, tag="ps", name="ps_s")
                for ci in range(CC):
                    mm(ps_s[:], hn[:, ci, bass.ts(mo, 128)],
                       kqt[:, ci, bass.ts(h, 512)], ci == 0, ci == CC - 1)
                ech = epool.tile([128, 512], rdt, tag="e", name="ech")
                if use_qkb:
                    nc.scalar.activation(out=ech[:], in_=ps_s[:], func=AF.Exp,
                                         scale=SCALE, bias=rt[:, mo, :])
                else:
                    nc.scalar.activation(out=ech[:], in_=ps_s[:], func=AF.Exp,
                                         scale=SCALE)
                return ech

            sched = [(h, mo) for h in range(NH) for mo in range(NM)]
            emitted = {}
            ptr = [0]

            def ensure(upto):
                while ptr[0] < len(sched) and ptr[0] <= upto:
                    hh, mm_ = sched[ptr[0]]
                    emitted[(hh, mm_)] = emit_scores(hh, mm_)
                    ptr[0] += 1

            def tail(h, ps_ua, ps_ub, zacc, ech_last):
                sl = bass.ts(h, 512)
                for co in range(CC):
                    pu = ps_ua if co < 2 else ps_ub
                    nc.scalar.activation(out=U[:, co, sl],
                                         in_=pu[:, co % 2, :], func=AF.Copy)
                zp = rpool.tile([128, 512], F32, tag="zp", name="zp")
                nc.gpsimd.partition_all_reduce(
                    zp[:], ech_last[:].bitcast(F32), channels=128,
                    reduce_op=bass_isa.ReduceOp.add)
                nc.vector.tensor_add(out=zacc[:], in0=zacc[:], in1=zp[:])
                nc.vector.reciprocal(out=Zb[:, sl], in_=zacc[:])
                if fuse_xt_b is not None:
                    # last batch: per-half epilogue fused right here so h0's
                    # normalize/store overlaps h1's matmuls
                    fxt, fb = fuse_xt_b
                    out_sb = opool.tile([128, CC, N], F32, tag="out",
                                        name="out_sb")
                    for co in range(CC):
                        nc.vector.tensor_mul(out=out_sb[:, co, sl],
                                             in0=U[:, co, sl], in1=Zb[:, sl])
                        nc.vector.tensor_add(out=out_sb[:, co, sl],
                                             in0=out_sb[:, co, sl],
                                             in1=fxt[:, co, sl])
                        if use_beff:
                            nc.vector.tensor_scalar_add(
                                out=out_sb[:, co, sl],
                                in0=out_sb[:, co, sl],
                                scalar1=beff_sb[:, co, :])
                        eng = nc.sync if co % 2 == 0 else nc.gpsimd
                        eng.dma_start(
                            out=outd.ap()[fb].rearrange(
                                "(cc p) n -> p cc n",
                                p=128)[:, co:co + 1, sl],
                            in_=out_sb[:, co:co + 1, sl])

            ensure(1)
            for h in range(NH):
                zacc = rpool.tile([128, 512], F32, tag="zacc", name="zacc")
                ps_ua = ps_ub = None
                ech_last = None
                for mo in range(NM):
                    i = h * NM + mo
                    ensure(i + 1)
                    ech = emitted.pop((h, mo))
                    if mo == 0:
                        ps_ua = acc_pool.tile([128, 2, 512], F32, tag="acca",
                                              name="ps_ua")
                        ps_ub = acc_pool.tile([128, 2, 512], F32, tag="accb",
                                              name="ps_ub")
                        nc.gpsimd.partition_all_reduce(
                            zacc[:], ech[:].bitcast(F32), channels=128,
                            reduce_op=bass_isa.ReduceOp.add)
                    elif mo < NM - 1:
                        zp = rpool.tile([128, 512], F32, tag="zp", name="zp")
                        nc.gpsimd.partition_all_reduce(
                            zp[:], ech[:].bitcast(F32), channels=128,
                            reduce_op=bass_isa.ReduceOp.add)
                        nc.vector.tensor_add(out=zacc[:], in0=zacc[:],
                                             in1=zp[:])
                    else:
                        ech_last = ech  # z-accumulate deferred past U copies
                    for co in range(CC):
                        pu = ps_ua if co < 2 else ps_ub
                        mm(pu[:, co % 2, :], vT[:, mo, bass.ts(co, 128)],
                           ech[:], mo == 0, mo == NM - 1)
                    if h == 0 and next_xt is not None:
                        stat_op(next_xt, sums_next, hn_next, mo)
                    if h == 1 and next_xt is not None:
                        if mo == 0:
                            ps_g = gn_sum_mms(sums_next)
                            gn_finish(ps_g)
                        elif mo == 2:
                            ab_next[0] = gn_ab(dve=True)
                        elif mo >= 4:
                            gn_hn_apply(next_xt, ab_next[0], hn_next, mo - 4)
                tail(h, ps_ua, ps_ub, zacc, ech_last)
            return U, Zb, hn_next

        def epilogue(U, Zb, xt, b):
            """Deferred normalize + residual + store (DVE work, emitted in
            the next batch's qkv window where DVE is otherwise light)."""
            out_sb = opool.tile([128, CC, N], F32, tag="out")
            for h in range(NH):
                sl = bass.ts(h, 512)
                for co in range(CC):
                    nc.vector.tensor_mul(out=out_sb[:, co, sl],
                                         in0=U[:, co, sl], in1=Zb[:, sl])
                    nc.vector.tensor_add(out=out_sb[:, co, sl],
                                         in0=out_sb[:, co, sl],
                                         in1=xt[:, co, sl])
                    if use_beff:
                        nc.vector.tensor_scalar_add(out=out_sb[:, co, sl],
                                                    in0=out_sb[:, co, sl],
                                                    scalar1=beff_sb[:, co, :])
                    eng = nc.sync if co % 2 == 0 else nc.gpsimd
                    eng.dma_start(
                        out=outd.ap()[b].rearrange(
                            "(cc p) n -> p cc n", p=128)[:, co:co + 1, sl],
                        in_=out_sb[:, co:co + 1, sl])

        # ---- software-pipelined batch loop ----
        pending = None
        xt_cur = xt0
        hn_cur = None
        for b in range(nbatch):
            if b == 0:
                ps_g = gn_sum_mms(sums0, dve_chunks=b0_dve_chunks)
                gn_finish(ps_g)
                hn_cur = gn_apply(xt_cur, hn0)
            kqt, vT, rt = qkv(hn_cur)
            if pending is not None:
                epilogue(*pending)
            xt_next = load_x(b + 1) if b + 1 < nbatch else None
            last = b == nbatch - 1
            U, Zb, hn_next = attention(
                hn_cur, kqt, vT, rt, next_xt=xt_next,
                fuse_xt_b=(xt_cur, b) if last else None)
            if not last:
                pending = (U, Zb, xt_cur, b)
            xt_cur = xt_next
            hn_cur = hn_next

    nc.compile()
    return nc


def make_host_inputs(x, gn_scale, gn_bias, wq, bq, wk, bk, wv, bv, wo, bo,
                     n_cores=8):
    """Shard + precompute host-side arrays. Returns (in_maps, nbatch)."""
    B = x.shape[0]
    nbatch = B // n_cores
    xr = np.ascontiguousarray(np.asarray(x, np.float32).reshape(B, C, N))
    beff = (np.asarray(wo, np.float32) @ np.asarray(bv, np.float32)
            + np.asarray(bo, np.float32))
    vpack = np.zeros((C, VP), np.float32)
    vpack[:, 0] = np.asarray(gn_scale, np.float32)
    vpack[:, 1] = np.asarray(gn_bias, np.float32)
    vpack[:, 2] = np.asarray(bq, np.float32)
    vpack[:, 3] = np.asarray(bk, np.float32)
    vpack[:, 4] = beff
    cidx = np.arange(C)
    vpack[cidx, 5 + cidx // GW] = 1.0 / (GW * N)
    vpack[cidx, 21 + cidx // GW] = 1.0 / GW
    indT = np.zeros((33, C), np.float32)
    indT[cidx // GW, cidx] = np.asarray(gn_scale, np.float32)
    indT[32, :] = np.asarray(gn_bias, np.float32)
    wqf = np.asarray(wq, np.float32)
    wkf = np.asarray(wk, np.float32)
    common = {
        # q and k projections fold into one: scores = hn.T (wk.T wq) hn
        "wqkT": np.ascontiguousarray(wqf.T @ wkf),
        # m-dependent score bias from bq (zero-bias case: unused)
        "rvec": ((wkf.T @ np.asarray(bq, np.float32)).reshape(C, 1)
                 / np.sqrt(np.float32(C))).astype(np.float32),
        # wo is folded into the value projection: the attention-weighted sum
        # commutes with the (linear) out-projection
        "wvT": np.ascontiguousarray(
            (np.asarray(wo, np.float32) @ np.asarray(wv, np.float32)).T),
        "vpack": vpack,
        "indT": indT,
        "ones": np.ones((128, 1), np.float32),
    }
    in_maps = []
    for i in range(n_cores):
        m = dict(common)
        m["xs"] = np.ascontiguousarray(xr[i * nbatch:(i + 1) * nbatch])
        in_maps.append(m)
    return in_maps, nbatch


_NC_CACHE = {}


def _get_nc(nbatch, use_beff, use_qkb):
    key = (nbatch, use_beff, use_qkb)
    if key not in _NC_CACHE:
        _NC_CACHE[key] = build_attention_nc(nbatch=nbatch, mm_dt="f32r",
                                            n_cores=8, use_beff=use_beff,
                                            use_qkb=use_qkb)
    return _NC_CACHE[key]


def kernel(x, gn_scale, gn_bias, wq, bq, wk, bk, wv, bv, wo, bo):
    """Full-input entry point: shards over 8 NeuronCores, returns full out."""
    from concourse.bass_utils import run_bass_kernel_spmd

    x = np.asarray(x, np.float32)
    B, Cin, H, W = x.shape
    assert (Cin, H * W) == (C, N), f"unexpected shape {x.shape}"
    n_cores = 8
    assert B % n_cores == 0
    in_maps, nbatch = make_host_inputs(
        x.reshape(B, C, N), gn_scale, gn_bias, wq, bq, wk, bk, wv, bv, wo, bo,
        n_cores=n_cores)
    use_beff = bool(np.any(in_maps[0]["vpack"][:, 4]))
    use_qkb = bool(np.any(np.asarray(bq, np.float32)))
    nc = _get_nc(nbatch, use_beff, use_qkb)
    res = run_bass_kernel_spmd(nc, in_maps, core_ids=list(range(n_cores)))
    out = np.concatenate([res.results[i]["out"] for i in range(n_cores)],
                         axis=0)
    return out.reshape(B, Cin, H, W).astype(np.float32)


# revision 7
# speedup vs baseline: 1.3305x; 1.0018x over previous
"""Self-contained Trainium2 Bass kernel for nn_AttentionBlock_80315888435976.

AttentionBlock: GroupNorm(16 groups) -> 1x1-conv q/k/v -> softmax attention
over the 32x32 spatial grid -> 1x1-conv out-projection -> residual.
Input x: [32, 512, 32, 32] fp32; weights [512, 512]; all biases [512].

Distribution: data-parallel over the batch dim across 8 NeuronCores
(4 batch elements per core); weights broadcast; no collectives.

Algebraic folds (host-side): scores = hn.T (wk.T wq) hn (q/k projections
collapse into one; the n-dependent bias terms cancel inside softmax), and
the out-projection commutes with the attention-weighted sum so the value
matrix is (wo @ wv) and the U accumulation directly yields the projected
output. v/out biases fold to a single per-channel constant wo@bv+bo.
"""
import sys
sys.path.insert(0, "/opt/trn_rl_repo")

import contextlib
import numpy as np

import concourse.bass as bass
import concourse.bass_isa as bass_isa
import concourse.bacc as bacc
import concourse.tile as tile
from concourse import mybir

F32 = mybir.dt.float32
F32R = mybir.dt.float32r
U32 = mybir.dt.uint32
AF = mybir.ActivationFunctionType
OP = mybir.AluOpType

C = 512
N = 1024
G = 16
GW = C // G      # 32 channels per group
CC = C // 128    # 4 channel chunks
NM = N // 128    # 8 m chunks
NH = N // 512    # 2 free halves
EPS = 1e-6
SCALE = 1.0 / np.sqrt(C)
# vecpack columns: 0 gnsc, 1 gnb, 2 bq, 3 bk, 4 beff,
#                  5:21 indm_sums (1/(GW*N)), 21:37 indm_mv (1/GW)
VP = 37
GE = 33        # gse rows: 0..15 = groups, 32 = bias row (base-partition
               # alignment: compute-engine APs must start at multiples of 32)


def build_attention_nc(nbatch=4, mm_dt="f32r", n_cores=8, use_beff=False,
                       use_qkb=False):
    nc = bacc.Bacc("TRN2", target_bir_lowering=False, debug=False,
                   num_devices=n_cores)
    rdt = F32R if mm_dt == "f32r" else F32

    xs = nc.dram_tensor("xs", [nbatch, C, N], F32, kind="ExternalInput")
    wqk = nc.dram_tensor("wqkT", [C, C], rdt, kind="ExternalInput")
    wv = nc.dram_tensor("wvT", [C, C], rdt, kind="ExternalInput")
    rvec = nc.dram_tensor("rvec", [C, 1], rdt, kind="ExternalInput")
    vpack = nc.dram_tensor("vpack", [C, VP], F32, kind="ExternalInput")
    indT = nc.dram_tensor("indT", [GE, C], F32, kind="ExternalInput")
    onesd = nc.dram_tensor("ones", [128, 1], rdt, kind="ExternalInput")
    outd = nc.dram_tensor("out", [nbatch, C, N], F32, kind="ExternalOutput")

    def r(dram2d):  # [C, X] dram -> [128, CC, X] view
        return dram2d.ap().rearrange("(cc p) x -> p cc x", p=128)

    def mm(ps, lhsT, rhs, start, stop):
        nc.tensor.matmul(ps, lhsT, rhs, start=start, stop=stop)

    with tile.TileContext(nc) as tc, contextlib.ExitStack() as ctx:
        wpool = ctx.enter_context(tc.tile_pool(name="w", bufs=1))
        vecs = ctx.enter_context(tc.tile_pool(name="vecs", bufs=1))
        xpool = ctx.enter_context(tc.tile_pool(name="x", bufs=3))
        hpool = ctx.enter_context(tc.tile_pool(name="hn", bufs=2))
        qkpool = ctx.enter_context(tc.tile_pool(name="qk", bufs=1))
        vpool = ctx.enter_context(tc.tile_pool(name="v", bufs=1))
        epool = ctx.enter_context(tc.tile_pool(name="e", bufs=3))
        upool = ctx.enter_context(tc.tile_pool(name="u", bufs=1))
        opool = ctx.enter_context(tc.tile_pool(name="o", bufs=1))
        rpool = ctx.enter_context(tc.tile_pool(name="r", bufs=2))
        stats = ctx.enter_context(tc.tile_pool(name="st", bufs=2))
        ps_pool = ctx.enter_context(tc.tile_pool(name="ps", bufs=4, space="PSUM"))
        acc_pool = ctx.enter_context(tc.tile_pool(name="acc", bufs=1, space="PSUM"))

        # ---- constants (3 DMAs; HWDGE issue pipe costs ~0.65us per DMA) ----
        vp_sb = vecs.tile([128, CC, VP], F32, tag="vp")
        indT_sb = vecs.tile([GE, CC, 128], F32, tag="indT")
        ones_sb = vecs.tile([128, 1], rdt, tag="ones")
        gse = vecs.tile([GE, 2], F32, tag="gse")
        eps_sb = vecs.tile([G, 1], F32, tag="eps")
        magic_sb = vecs.tile([G, 1], U32, tag="magic")
        c15_sb = vecs.tile([G, 1], F32, tag="c15")
        nc.vector.memset(eps_sb[:], EPS)
        nc.vector.memset(magic_sb[:], 0x5f3759df)
        nc.vector.memset(c15_sb[:], 1.5)
        nc.vector.memset(gse[32:GE, 0:1], 0.0)
        nc.vector.memset(gse[32:GE, 1:2], 1.0)
        gnsc_sb = vp_sb[:, :, 0:1]
        gnb_sb = vp_sb[:, :, 1:2]
        bq_sb = vp_sb[:, :, 2:3]
        bk_sb = vp_sb[:, :, 3:4]
        beff_sb = vp_sb[:, :, 4:5]

        def stat_op(xt, sums, scr, k):
            """k-th of 8 ACT ops accumulating per-channel sum / sum-sq.
            scr is a scratch dummy output (only accum_out matters)."""
            cc, which = divmod(k, 2)
            nc.scalar.activation(out=scr[:, cc, :],
                                 in_=xt[:, cc, :],
                                 func=(AF.Copy if which == 0 else AF.Square),
                                 accum_out=sums[:, cc, which:which + 1])

        def gn_stat_tiles():
            # (scr is not allocated here: the next batch's hn tile doubles as
            # the dummy activation output until its real write in h1)
            return stats.tile([128, CC, 2], F32, tag="sums", name="sums")

        def gn_sum_mms(sums, dve_chunks=()):
            ps_g = ps_pool.tile([G, 2], F32, tag="ps")
            for cc in range(CC):
                col = slice(21, 37) if cc in dve_chunks else slice(5, 21)
                nc.tensor.matmul(ps_g[:], vp_sb[:, cc, col], sums[:, cc, :],
                                 start=(cc == 0), stop=(cc == CC - 1))
            return ps_g

        # ---- batch-0 x load: per chunk, stats split ACT/DVE ----
        xt0 = xpool.tile([128, CC, N], F32, tag="x", name="xt0")
        hn0 = hpool.tile([128, CC, N], F32R if mm_dt == "f32r" else F32,
                         tag="hn", name="hn0")
        sums0 = gn_stat_tiles()
        st6_0 = stats.tile([128, CC, 2, 6], F32, tag="st6")
        mv0 = stats.tile([128, CC, 2], F32, tag="mv")
        b0_dve_chunks = (1, 3)
        for cc in range(CC):
            nc.sync.dma_start(out=xt0[:, cc, :],
                              in_=xs.ap()[0][bass.ts(cc, 128), :])
            if cc in b0_dve_chunks:
                # DVE path -> sums0[:, cc] = [mu_c, mu_c^2 + var_c]
                for h in range(2):
                    nc.vector.bn_stats(out=st6_0[:, cc, h, :],
                                       in_=xt0[:, cc, bass.ts(h, 512)])
                nc.vector.bn_aggr(out=mv0[:, cc, :], in_=st6_0[:, cc, :, :])
                nc.vector.tensor_mul(out=sums0[:, cc, 1:2],
                                     in0=mv0[:, cc, 0:1], in1=mv0[:, cc, 0:1])
                nc.vector.tensor_add(out=sums0[:, cc, 1:2],
                                     in0=sums0[:, cc, 1:2], in1=mv0[:, cc, 1:2])
                nc.vector.tensor_copy(out=sums0[:, cc, 0:1],
                                      in_=mv0[:, cc, 0:1])
            else:
                stat_op(xt0, sums0, hn0, 2 * cc)
                stat_op(xt0, sums0, hn0, 2 * cc + 1)

        nc.sync.dma_start(out=vp_sb[:], in_=r(vpack))
        nc.sync.dma_start(
            out=indT_sb[:], in_=indT.ap().rearrange("g (cc p) -> g cc p", p=128))
        nc.sync.dma_start(out=ones_sb[:], in_=onesd.ap())

        wqk_sb = wpool.tile([128, CC, C], rdt, tag="wqk")
        wv_sb = wpool.tile([128, CC, C], rdt, tag="wv")
        nc.sync.dma_start(out=wqk_sb[:], in_=r(wqk))
        nc.sync.dma_start(out=wv_sb[:], in_=r(wv))
        rv_sb = None
        if use_qkb:
            rv_sb = vecs.tile([128, CC, 1], rdt, tag="rv")
            nc.sync.dma_start(out=rv_sb[:], in_=r(rvec))

        def load_x(b):
            xt = xpool.tile([128, CC, N], F32, tag="x")
            nc.sync.dma_start(
                out=xt[:], in_=xs.ap()[b].rearrange("(cc p) n -> p cc n", p=128))
            return xt

        def gn_finish(ps_g):
            """[mu_g, m2_g] -> gse rows 0..15 = [rstd_g, -mu_g*rstd_g]."""
            gsb = stats.tile([G, 2], F32, tag="gsb")
            varg = stats.tile([G, 1], F32, tag="varg")
            nc.vector.tensor_copy(out=gsb[:], in_=ps_g[:])
            nc.vector.tensor_mul(out=varg[:], in0=gsb[:, 0:1], in1=gsb[:, 0:1])
            nc.vector.tensor_tensor(out=varg[:], in0=gsb[:, 1:2], in1=varg[:],
                                    op=OP.subtract)
            nc.vector.tensor_scalar_add(out=varg[:], in0=varg[:], scalar1=EPS)
            y = stats.tile([G, 1], F32, tag="nwt_y")
            vh = stats.tile([G, 1], F32, tag="nwt_vh")
            t = stats.tile([G, 1], F32, tag="nwt_t")
            nc.vector.tensor_scalar(out=t[:].bitcast(U32),
                                    in0=varg[:].bitcast(U32),
                                    scalar1=1, scalar2=None,
                                    op0=OP.logical_shift_right)
            nc.vector.tensor_tensor(out=y[:].bitcast(U32), in0=magic_sb[:],
                                    in1=t[:].bitcast(U32), op=OP.subtract)
            nc.vector.tensor_scalar_mul(out=vh[:], in0=varg[:], scalar1=0.5)
            for it in range(2):
                nc.vector.tensor_mul(out=t[:], in0=y[:], in1=y[:])
                nc.vector.tensor_mul(out=t[:], in0=vh[:], in1=t[:])
                nc.vector.tensor_tensor(out=t[:], in0=c15_sb[:], in1=t[:],
                                        op=OP.subtract)
                dst = gse[0:G, 0:1] if it == 1 else y[:]
                nc.vector.tensor_mul(out=dst, in0=y[:], in1=t[:])
            nc.vector.tensor_mul(out=t[:], in0=gsb[:, 0:1], in1=gse[0:G, 0:1])
            nc.vector.tensor_scalar_mul(out=gse[0:G, 1:2], in0=t[:],
                                        scalar1=-1.0)

        def gn_ab(dve=False):
            ab_sb = stats.tile([128, CC, 2], F32, tag="ab_sb")
            for cc in range(CC):
                ps_cb = ps_pool.tile([128, 2], F32, tag="ps")
                nc.tensor.matmul(ps_cb[:], indT_sb[:, cc, :], gse[:],
                                 start=True, stop=True)
                if dve:
                    nc.vector.tensor_copy(out=ab_sb[:, cc, :], in_=ps_cb[:])
                else:
                    nc.scalar.activation(out=ab_sb[:, cc, :], in_=ps_cb[:],
                                         func=AF.Copy)
            return ab_sb

        def gn_hn_apply(xt, ab_sb, hn, cc, dve_extra=False):
            if cc % 2 or (dve_extra and cc == 2):
                nc.vector.tensor_scalar(out=hn[:, cc, :], in0=xt[:, cc, :],
                                        scalar1=ab_sb[:, cc, 0:1],
                                        scalar2=ab_sb[:, cc, 1:2],
                                        op0=OP.mult, op1=OP.add)
            else:
                nc.scalar.activation(out=hn[:, cc, :], in_=xt[:, cc, :],
                                     func=AF.Identity,
                                     scale=ab_sb[:, cc, 0:1],
                                     bias=ab_sb[:, cc, 1:2])

        def gn_apply(xt, hn):
            ab_sb = gn_ab(dve=True)
            for cc in range(CC):
                gn_hn_apply(xt, ab_sb, hn, cc)
            return hn

        def qkv(hn):
            """kq = (wk.T wq) @ hn  (q and k fold into one projection: the
            softmax over m is invariant to per-n additive constants).
            vT = hn.T @ (wo@wv).T.  With nonzero bq, the m-dependent score
            bias r[m] = (wk.T bq).hn[:,m] is accumulated for use as a
            per-partition exp bias."""
            kqt = qkpool.tile([128, CC, N], rdt, tag="kq")
            for co in range(CC):
                for h in range(NH):
                    ps_t = ps_pool.tile([128, 512], F32, tag="ps")
                    for ci in range(CC):
                        mm(ps_t[:], wqk_sb[:, ci, bass.ts(co, 128)],
                           hn[:, ci, bass.ts(h, 512)], ci == 0, ci == CC - 1)
                    if (co + h) % 2:
                        nc.scalar.activation(out=kqt[:, co, bass.ts(h, 512)],
                                             in_=ps_t[:], func=AF.Copy)
                    else:
                        nc.vector.tensor_copy(
                            out=kqt[:, co, bass.ts(h, 512)], in_=ps_t[:])
            vT = vpool.tile([128, NM, C], rdt, tag="vT")
            rt = None
            if use_qkb:
                rt = stats.tile([128, NM, 1], F32, tag="rt")
            for mo in range(NM):
                ps_t = ps_pool.tile([128, 512], F32, tag="ps")
                for ci in range(CC):
                    mm(ps_t[:], hn[:, ci, bass.ts(mo, 128)], wv_sb[:, ci, :],
                       ci == 0, ci == CC - 1)
                if mo % 2:
                    nc.scalar.activation(out=vT[:, mo, :], in_=ps_t[:],
                                         func=AF.Copy)
                else:
                    nc.vector.tensor_copy(out=vT[:, mo, :], in_=ps_t[:])
                if use_qkb:
                    ps_r = ps_pool.tile([128, 1], F32, tag="ps")
                    for ci in range(CC):
                        nc.tensor.matmul(ps_r[:],
                                         hn[:, ci, bass.ts(mo, 128)],
                                         rv_sb[:, ci, :],
                                         start=(ci == 0), stop=(ci == CC - 1))
                    nc.vector.tensor_copy(out=rt[:, mo, :], in_=ps_r[:])
            return kqt, vT, rt

        def attention(hn, kqt, vT, rt, next_xt=None, fuse_xt_b=None):
            """vT is hn.T @ (wo@wv).T: the U accumulation directly yields the
            unnormalized out-projection; normalize/residual run deferred in
            the next batch's qkv window (see epilogue)."""
            Zb = rpool.tile([128, N], F32, tag="Zb")
            U = upool.tile([128, CC, N], rdt, tag="U")
            sums_next = hn_next = None
            ab_next = [None]
            if next_xt is not None:
                sums_next = gn_stat_tiles()
                hn_next = hpool.tile([128, CC, N], rdt, tag="hn", name="hn")

            def emit_scores(h, mo):
                ps_s = ps_pool.tile([128, 512], F32, tag="ps", name="ps_s")
                for ci in range(CC):
                    mm(ps_s[:], hn[:, ci, bass.ts(mo, 128)],
                       kqt[:, ci, bass.ts(h, 512)], ci == 0, ci == CC - 1)
                ech = epool.tile([128, 512], rdt, tag="e", name="ech")
                if use_qkb:
                    nc.scalar.activation(out=ech[:], in_=ps_s[:], func=AF.Exp,
                                         scale=SCALE, bias=rt[:, mo, :])
                else:
                    nc.scalar.activation(out=ech[:], in_=ps_s[:], func=AF.Exp,
                                         scale=SCALE)
                return ech

            sched = [(h, mo) for h in range(NH) for mo in range(NM)]
            emitted = {}
            ptr = [0]

            def ensure(upto):
                while ptr[0] < len(sched) and ptr[0] <= upto:
                    hh, mm_ = sched[ptr[0]]
                    emitted[(hh, mm_)] = emit_scores(hh, mm_)
                    ptr[0] += 1

            def tail(h, ps_ua, ps_ub, zacc, ech_last):
                sl = bass.ts(h, 512)
                for co in range(CC):
                    pu = ps_ua if co < 2 else ps_ub
                    nc.scalar.activation(out=U[:, co, sl],
                                         in_=pu[:, co % 2, :], func=AF.Copy)
                zp = rpool.tile([128, 512], F32, tag="zp", name="zp")
                nc.gpsimd.partition_all_reduce(
                    zp[:], ech_last[:].bitcast(F32), channels=128,
                    reduce_op=bass_isa.ReduceOp.add)
                nc.vector.tensor_add(out=zacc[:], in0=zacc[:], in1=zp[:])
                nc.vector.reciprocal(out=Zb[:, sl], in_=zacc[:])
                if fuse_xt_b is not None:
                    # last batch: per-half epilogue fused right here so h0's
                    # normalize/store overlaps h1's matmuls
                    fxt, fb = fuse_xt_b
                    out_sb = opool.tile([128, CC, N], F32, tag="out",
                                        name="out_sb")
                    for co in range(CC):
                        nc.vector.tensor_mul(out=out_sb[:, co, sl],
                                             in0=U[:, co, sl], in1=Zb[:, sl])
                        nc.vector.tensor_add(out=out_sb[:, co, sl],
                                             in0=out_sb[:, co, sl],
                                             in1=fxt[:, co, sl])
                        if use_beff:
                            nc.vector.tensor_scalar_add(
                                out=out_sb[:, co, sl],
                                in0=out_sb[:, co, sl],
                                scalar1=beff_sb[:, co, :])
                        eng = nc.sync if co % 2 == 0 else nc.gpsimd
                        eng.dma_start(
                            out=outd.ap()[fb].rearrange(
                                "(cc p) n -> p cc n",
                                p=128)[:, co:co + 1, sl],
                            in_=out_sb[:, co:co + 1, sl])

            ensure(1)
            for h in range(NH):
                zacc = rpool.tile([128, 512], F32, tag="zacc", name="zacc")
                ps_ua = ps_ub = None
                ech_last = None
                for mo in range(NM):
                    i = h * NM + mo
                    ensure(i + 1)
                    ech = emitted.pop((h, mo))
                    if mo == 0:
                        ps_ua = acc_pool.tile([128, 2, 512], F32, tag="acca",
                                              name="ps_ua")
                        ps_ub = acc_pool.tile([128, 2, 512], F32, tag="accb",
                                              name="ps_ub")
                        nc.gpsimd.partition_all_reduce(
                            zacc[:], ech[:].bitcast(F32), channels=128,
                            reduce_op=bass_isa.ReduceOp.add)
                    elif mo < NM - 1:
                        zp = rpool.tile([128, 512], F32, tag="zp", name="zp")
                        nc.gpsimd.partition_all_reduce(
                            zp[:], ech[:].bitcast(F32), channels=128,
                            reduce_op=bass_isa.ReduceOp.add)
                        nc.vector.tensor_add(out=zacc[:], in0=zacc[:],
                                             in1=zp[:])
                    else:
                        ech_last = ech  # z-accumulate deferred past U copies
                    for co in range(CC):
                        pu = ps_ua if co < 2 else ps_ub
                        mm(pu[:, co % 2, :], vT[:, mo, bass.ts(co, 128)],
                           ech[:], mo == 0, mo == NM - 1)
                    if h == 0 and next_xt is not None:
                        stat_op(next_xt, sums_next, hn_next, mo)
                    if h == 1 and next_xt is not None:
                        if mo == 0:
                            ps_g = gn_sum_mms(sums_next)
                            gn_finish(ps_g)
                        elif mo == 2:
                            ab_next[0] = gn_ab(dve=True)
                        elif mo >= 4:
                            gn_hn_apply(next_xt, ab_next[0], hn_next, mo - 4)
                tail(h, ps_ua, ps_ub, zacc, ech_last)
            return U, Zb, hn_next

        def epilogue(U, Zb, xt, b):
            """Deferred normalize + residual + store (DVE work, emitted in
            the next batch's qkv window where DVE is otherwise light)."""
            out_sb = opool.tile([128, CC, N], F32, tag="out")
            for h in range(NH):
                sl = bass.ts(h, 512)
                for co in range(CC):
                    nc.vector.tensor_mul(out=out_sb[:, co, sl],
                                         in0=U[:, co, sl], in1=Zb[:, sl])
                    nc.vector.tensor_add(out=out_sb[:, co, sl],
                                         in0=out_sb[:, co, sl],
                                         in1=xt[:, co, sl])
                    if use_beff:
                        nc.vector.tensor_scalar_add(out=out_sb[:, co, sl],
                                                    in0=out_sb[:, co, sl],
                                                    scalar1=beff_sb[:, co, :])
                    eng = nc.sync if co % 2 == 0 else nc.gpsimd
                    eng.dma_start(
                        out=outd.ap()[b].rearrange(
                            "(cc p) n -> p cc n", p=128)[:, co:co + 1, sl],
                        in_=out_sb[:, co:co + 1, sl])

        # ---- software-pipelined batch loop ----
        pending = None
        xt_cur = xt0
        hn_cur = None
        for b in range(nbatch):
            if b == 0:
                ps_g = gn_sum_mms(sums0, dve_chunks=b0_dve_chunks)
                gn_finish(ps_g)
                hn_cur = gn_apply(xt_cur, hn0)
            kqt, vT, rt = qkv(hn_cur)
            if pending is not None:
                epilogue(*pending)
            xt_next = load_x(b + 1) if b + 1 < nbatch else None
            last = b == nbatch - 1
            U, Zb, hn_next = attention(
                hn_cur, kqt, vT, rt, next_xt=xt_next,
                fuse_xt_b=(xt_cur, b) if last else None)
            if not last:
                pending = (U, Zb, xt_cur, b)
            xt_cur = xt_next
            hn_cur = hn_next

    nc.compile()
    return nc


def make_host_inputs(x, gn_scale, gn_bias, wq, bq, wk, bk, wv, bv, wo, bo,
                     n_cores=8):
    """Shard + precompute host-side arrays. Returns (in_maps, nbatch)."""
    B = x.shape[0]
    nbatch = B // n_cores
    xr = np.ascontiguousarray(np.asarray(x, np.float32).reshape(B, C, N))
    beff = (np.asarray(wo, np.float32) @ np.asarray(bv, np.float32)
            + np.asarray(bo, np.float32))
    vpack = np.zeros((C, VP), np.float32)
    vpack[:, 0] = np.asarray(gn_scale, np.float32)
    vpack[:, 1] = np.asarray(gn_bias, np.float32)
    vpack[:, 2] = np.asarray(bq, np.float32)
    vpack[:, 3] = np.asarray(bk, np.float32)
    vpack[:, 4] = beff
    cidx = np.arange(C)
    vpack[cidx, 5 + cidx // GW] = 1.0 / (GW * N)
    vpack[cidx, 21 + cidx // GW] = 1.0 / GW
    indT = np.zeros((33, C), np.float32)
    indT[cidx // GW, cidx] = np.asarray(gn_scale, np.float32)
    indT[32, :] = np.asarray(gn_bias, np.float32)
    wqf = np.asarray(wq, np.float32)
    wkf = np.asarray(wk, np.float32)
    common = {
        # q and k projections fold into one: scores = hn.T (wk.T wq) hn
        "wqkT": np.ascontiguousarray(wqf.T @ wkf),
        # m-dependent score bias from bq (zero-bias case: unused)
        "rvec": ((wkf.T @ np.asarray(bq, np.float32)).reshape(C, 1)
                 / np.sqrt(np.float32(C))).astype(np.float32),
        # wo is folded into the value projection: the attention-weighted sum
        # commutes with the (linear) out-projection
        "wvT": np.ascontiguousarray(
            (np.asarray(wo, np.float32) @ np.asarray(wv, np.float32)).T),
        "vpack": vpack,
        "indT": indT,
        "ones": np.ones((128, 1), np.float32),
    }
    in_maps = []
    for i in range(n_cores):
        m = dict(common)
        m["xs"] = np.ascontiguousarray(xr[i * nbatch:(i + 1) * nbatch])
        in_maps.append(m)
    return in_maps, nbatch


_NC_CACHE = {}


def _get_nc(nbatch, use_beff, use_qkb):
    key = (nbatch, use_beff, use_qkb)
    if key not in _NC_CACHE:
        _NC_CACHE[key] = build_attention_nc(nbatch=nbatch, mm_dt="f32r",
                                            n_cores=8, use_beff=use_beff,
                                            use_qkb=use_qkb)
    return _NC_CACHE[key]


def kernel(x, gn_scale, gn_bias, wq, bq, wk, bk, wv, bv, wo, bo):
    """Full-input entry point: shards over 8 NeuronCores, returns full out."""
    from concourse.bass_utils import run_bass_kernel_spmd

    x = np.asarray(x, np.float32)
    B, Cin, H, W = x.shape
    assert (Cin, H * W) == (C, N), f"unexpected shape {x.shape}"
    n_cores = 8
    assert B % n_cores == 0
    in_maps, nbatch = make_host_inputs(
        x.reshape(B, C, N), gn_scale, gn_bias, wq, bq, wk, bk, wv, bv, wo, bo,
        n_cores=n_cores)
    use_beff = bool(np.any(in_maps[0]["vpack"][:, 4]))
    use_qkb = bool(np.any(np.asarray(bq, np.float32)))
    nc = _get_nc(nbatch, use_beff, use_qkb)
    res = run_bass_kernel_spmd(nc, in_maps, core_ids=list(range(n_cores)))
    out = np.concatenate([res.results[i]["out"] for i in range(n_cores)],
                         axis=0)
    return out.reshape(B, Cin, H, W).astype(np.float32)
